# revision 12
# baseline (speedup 1.0000x reference)
"""MoE-routed K-cluster autoencoder kernel for 8 Trainium2 NeuronCores.

Strategy
--------
Each row of x is reconstructed by the autoencoder of its kmeans cluster.
Computing all K experts densely for every row (like the reference) does
10x the needed matmul work, so we *route*.

Structured path (default for ~uniform labels): the label histogram is
known at call time, so slot capacities are fitted to it.  The two
largest clusters are split 4 ways into the per-core "col4" slot (4 cores
each); the remaining 8 clusters are each *owned* by one core and span
that core's slots 0-2, which share a single weight load.  Per-core slot
capacities (1024, 1024, M-2048, ceil(maxbig/4)) give ~4170 row-slots vs
4608 for the old fixed-capacity config (-10% PE work and x/y bytes), and
owning one cluster per core halves the weight DMA (2 unique clusters per
core instead of 4).  Chunks are ~512 columns (one PSUM bank) to minimize
LDWEIGHTS re-issues and PSUM->SBUF eviction instruction count.  The
decoder output is drained stripe-major and each 112-feature stripe is
DMA'd out as soon as it is evicted, so the final slot exposes only one
stripe's DMA instead of the whole slot.  Slot 0's first x chunk lands in
two k-split DMAs so e0 can start after ~half the chunk arrives.

Fallback path (skewed/degenerate labels): the original fixed-capacity
slot config search, unchanged.

Device pipeline (both paths): per slot, the 6-layer MLP chain runs as
feature-major matmuls (outT = W.T @ actsT); the next slot's encoder-0
work is software-pipelined into the current slot's serial mid-layers;
PSUM->SBUF bias+ReLU evictions alternate between ScalarE and VectorE; a
short pre-warm matmul burst opens the HAM clock gate while the first
DMAs land.  bf16 operands end-to-end (~5.6e-3 scale-relative error).
"""

import numpy as np

import concourse.tile as tile
from concourse import bacc, mybir
from concourse.bass_utils import run_bass_kernel_spmd

N_CORES = 8
B, D, H1, H2, L, K = 32768, 784, 256, 64, 16, 10
P = 112          # partition tile height for the D axis: 784 = 7 * 112
KT = D // P      # 7 k-tiles along D

# packed weight layout (column offsets in a [128, WSLOT] block)
_E0, _E1, _E2, _D0, _D1, _D2 = 0, 1792, 1920, 1936, 2000, 2256
WSLOT = 3824     # = 7*256 + 2*64 + 16 + 64 + 256 + 2*784
BSLOT = 14       # bias columns per block: 2 + 1 + 1 + 1 + 2 + 7

# (slots_per_core, rows_per_slot) fallback configs
_CONFIGS = [(4, 1152), (4, 1280), (8, 640), (16, 320), (32, 160)]

_F32 = mybir.dt.float32
_F32R = mybir.dt.float32r
_BF16 = mybir.dt.bfloat16
_RELU = mybir.ActivationFunctionType.Relu

MODE = "bf16"


def _slot_chunks(C):
    """Split C columns into equal-ish chunks of <=512 (one PSUM bank)."""
    n = max(1, (C + 511) // 512)
    base, extra = divmod(C, n)
    return [base + (1 if i < extra else 0) for i in range(n)]


# ---------------------------------------------------------------------------
# structured program: caps per slot, slots 0-2 share weight block 0,
# slot 3 uses weight block 1.  x chunk-flattened; y stripe-major.
# ---------------------------------------------------------------------------

def _build_program_structured(caps):
    S = len(caps)
    chunk_lists = [_slot_chunks(c) for c in caps]
    nflat = KT * sum(caps)
    nc = bacc.Bacc("TRN2", target_bir_lowering=False, debug=False)
    xt = nc.dram_tensor("xt", [P, nflat], _BF16, kind="ExternalInput").ap()
    wp = nc.dram_tensor("wp", [128, 2 * WSLOT], _BF16, kind="ExternalInput").ap()
    bp = nc.dram_tensor("bp", [128, 2 * BSLOT], _F32, kind="ExternalInput").ap()
    yt = nc.dram_tensor("yt", [P, nflat], _BF16, kind="ExternalOutput").ap()

    # per-slot x column offsets (in xt/yt, units of columns)
    slot_off = []
    cum = 0
    for c in caps:
        slot_off.append(cum)
        cum += KT * c

    # slot -> weight/bias block: slots 0-2 share the owned cluster's
    # weights; the remaining slot(s) hold the big cluster's
    wblk = [0, 0, 0] + [1] * (S - 3)

    with tile.TileContext(nc) as tc:
        with (
            tc.tile_pool(name="wpool", bufs=1) as wpool,
            tc.tile_pool(name="iopool", bufs=1) as iopool,
            tc.tile_pool(name="apool", bufs=1) as apool,
            tc.tile_pool(name="pspool", bufs=1, space="PSUM") as pspool,
        ):
            bsb = wpool.tile([128, 2 * BSLOT], _F32, tag="b", name="bsb", bufs=1)
            wA = wpool.tile([128, WSLOT], _BF16, tag="wA", name="wA", bufs=1)
            wB = wpool.tile([128, WSLOT], _BF16, tag="wB", name="wB", bufs=1)

            xq = {}  # (s, ci) -> tile

            def fetch_slot_x(s, eng, ksplit=False):
                if s >= S:
                    return
                off = slot_off[s]
                for ci, nch in enumerate(chunk_lists[s]):
                    t = iopool.tile([128, KT, nch], _BF16, tag="xq",
                                    name="xq", bufs=7)
                    src = xt[:, off:off + KT * nch].rearrange(
                        "p (k n) -> p k n", k=KT)
                    if ksplit and ci == 0:
                        # subtile deps let e0's k<4 matmuls start after
                        # the first half of the chunk lands
                        eng.dma_start(out=t[0:P, 0:4], in_=src[:, 0:4])
                        eng.dma_start(out=t[0:P, 4:KT], in_=src[:, 4:KT])
                    else:
                        eng.dma_start(out=t[0:P], in_=src)
                    xq[(s, ci)] = t
                    off += KT * nch

            # head DMAs split across the two HWDGE engines so the enqueues
            # (~0.6us each, serialized per engine) don't gate the first e0
            fetch_slot_x(0, nc.sync, ksplit=True)
            nc.scalar.dma_start(out=wA[:, 0:_E1], in_=wp[:, 0:_E1])
            nc.sync.dma_start(out=bsb, in_=bp)
            nc.scalar.dma_start(out=wA[:, _E1:], in_=wp[:, _E1:WSLOT])
            fetch_slot_x(1, nc.sync)

            # PE pre-warm: a few throwaway matmuls get PE activity started
            # (HAM clock-gate) while the first DMAs land, without delaying
            # e0 behind a long in-order warmup burst.
            wu = wpool.tile([128, 512], _BF16, tag="wu", name="wu", bufs=1)
            nc.gpsimd.memset(wu, 0)
            wups = [pspool.tile([128, 512], _F32, tag="ps", name="wups",
                                bufs=8) for _ in range(4)]
            for i in range(6):
                nc.tensor.matmul(wups[i % 4], wu[:, 0:128], wu,
                                 start=True, stop=True)

            def wt(s):
                return wA if wblk[s] == 0 else wB

            def bias(s, lo, col):
                bb = wblk[s] * BSLOT
                return bsb[0:lo, bb + col:bb + col + 1]

            def ps_tile(parts, nch):
                return pspool.tile([parts, nch], _F32, tag="ps", name="ps",
                                   bufs=8)

            drain_i = [0]

            def drain_relu(out, ps, bias_ap):
                drain_i[0] += 1
                if drain_i[0] % 2:
                    nc.scalar.activation(out, ps, _RELU, bias=bias_ap)
                else:
                    nc.vector.tensor_scalar(out, ps, bias_ap, 0.0,
                                            mybir.AluOpType.add,
                                            mybir.AluOpType.max)

            def drain_bias(out, ps, bias_ap):
                drain_i[0] += 1
                if drain_i[0] % 2:
                    nc.scalar.add(out, ps, bias_ap)
                else:
                    nc.vector.tensor_scalar_add(out, ps, bias_ap)

            h1s = {}   # s -> [m][ci] h1 tiles
            a2s = {}   # s -> [m][ci] a2 tiles

            def e0_unit(s, ci, m):
                """One e0 unit: chunk ci, m-half m, all 7 k accumulations."""
                nch = chunk_lists[s][ci]
                w = wt(s)
                ps = ps_tile(128, nch)
                for k in range(KT):
                    wk = w[0:P, _E0 + k * 256 + 128 * m:
                           _E0 + k * 256 + 128 * m + 128]
                    nc.tensor.matmul(ps, wk, xq[(s, ci)][0:P, k, :],
                                     start=(k == 0), stop=(k == KT - 1))
                t = apool.tile([128, nch], _BF16, tag="h1", name="h1",
                               bufs=14)
                drain_relu(t, ps, bias(s, 128, m))
                h1s.setdefault(s, [{}, {}])[m][ci] = t

            def mid_steps(s):
                """The serial e1->e2->d0->d1 chain of slot s as 5 steps.

                Consecutive steps must be separated by ~1us of unrelated
                PE work (a d2 stripe of the previous slot) to hide the
                PSUM-eviction latency between layers."""
                if s >= S:
                    return
                chunks = chunk_lists[s]
                NCH = len(chunks)
                w = wt(s)
                st = {}

                def e1():
                    h1 = h1s[s]
                    ps = [None] * NCH
                    for k in range(2):
                        wk = w[0:128, _E1 + 64 * k:_E1 + 64 * k + 64]
                        for ci, nch in enumerate(chunks):
                            if k == 0:
                                ps[ci] = ps_tile(64, nch)
                            nc.tensor.matmul(ps[ci], wk, h1[k][ci],
                                             start=(k == 0), stop=(k == 1))
                    st["h2"] = []
                    for ci, nch in enumerate(chunks):
                        t = apool.tile([64, nch], _BF16, tag="h2", name="h2",
                                       bufs=4)
                        drain_relu(t, ps[ci], bias(s, 64, 2))
                        st["h2"].append(t)

                def e2():
                    ps = [None] * NCH
                    wk = w[0:64, _E2:_E2 + 16]
                    for ci, nch in enumerate(chunks):
                        ps[ci] = ps_tile(16, nch)
                        nc.tensor.matmul(ps[ci], wk, st["h2"][ci],
                                         start=True, stop=True)
                    st["z"] = []
                    for ci, nch in enumerate(chunks):
                        t = apool.tile([16, nch], _BF16, tag="z", name="z",
                                       bufs=4)
                        drain_relu(t, ps[ci], bias(s, 16, 3))
                        st["z"].append(t)

                def d0():
                    ps = [None] * NCH
                    wk = w[0:16, _D0:_D0 + 64]
                    for ci, nch in enumerate(chunks):
                        ps[ci] = ps_tile(64, nch)
                        nc.tensor.matmul(ps[ci], wk, st["z"][ci],
                                         start=True, stop=True)
                    st["a1"] = []
                    for ci, nch in enumerate(chunks):
                        t = apool.tile([64, nch], _BF16, tag="a1", name="a1",
                                       bufs=4)
                        drain_relu(t, ps[ci], bias(s, 64, 4))
                        st["a1"].append(t)

                def d1(m):
                    a2 = a2s.setdefault(s, [[None] * NCH, [None] * NCH])
                    wk = w[0:64, _D1 + 128 * m:_D1 + 128 * m + 128]
                    ps = [None] * NCH
                    for ci, nch in enumerate(chunks):
                        ps[ci] = ps_tile(128, nch)
                        nc.tensor.matmul(ps[ci], wk, st["a1"][ci],
                                         start=True, stop=True)
                    for ci, nch in enumerate(chunks):
                        t = apool.tile([128, nch], _BF16, tag="a2", name="a2",
                                       bufs=13)
                        drain_relu(t, ps[ci], bias(s, 128, 5 + m))
                        a2[m][ci] = t

                yield e1
                yield e2
                yield d0
                yield lambda: d1(0)
                yield lambda: d1(1)

            def e0_unit_steps(s):
                if s >= S:
                    return
                for ci in range(len(chunk_lists[s])):
                    for m in range(2):
                        yield (lambda ci=ci, m=m: e0_unit(s, ci, m))

            # ---- prologue: e0(0) inline; mid(0) interleaved with e0(1)
            for ci in range(len(chunk_lists[0])):
                for m in range(2):
                    e0_unit(0, ci, m)
            u1 = e0_unit_steps(1)
            for step in mid_steps(0):
                step()
                u = next(u1, None)
                if u is not None:
                    u()
            for u in u1:
                u()

            nc.scalar.dma_start(out=wB[:, 0:_E1],
                                in_=wp[:, WSLOT:WSLOT + _E1])
            nc.scalar.dma_start(out=wB[:, _E1:],
                                in_=wp[:, WSLOT + _E1:2 * WSLOT])

            # ---- main pipeline: iteration i runs d2(i) stripes with
            # mid(i+1) steps and e0(i+2) units interleaved between stripes
            for i in range(S):
                chunks = chunk_lists[i]
                NCH = len(chunks)
                w = wt(i)
                fetch_slot_x(i + 2, nc.scalar)
                steps = list(mid_steps(i + 1))
                units = list(e0_unit_steps(i + 2))
                a2 = a2s[i]
                col_off = []
                c0 = 0
                for nch in chunks:
                    col_off.append(c0)
                    c0 += nch
                Cs = caps[i]
                for mm in range(KT):
                    yqs = iopool.tile([112, Cs], _BF16, tag="yq", name="yqs",
                                      bufs=8)
                    ps = [None] * NCH
                    for k in range(2):
                        wk = w[0:128, _D2 + 784 * k + 112 * mm:
                               _D2 + 784 * k + 112 * mm + 112]
                        for ci, nch in enumerate(chunks):
                            if k == 0:
                                ps[ci] = ps_tile(112, nch)
                            nc.tensor.matmul(ps[ci], wk, a2[k][ci],
                                             start=(k == 0), stop=(k == 1))
                    for ci, nch in enumerate(chunks):
                        drain_bias(yqs[0:P, col_off[ci]:col_off[ci] + nch],
                                   ps[ci], bias(i, 112, 7 + mm))
                    nc.sync.dma_start(
                        out=yt[:, slot_off[i] + mm * Cs:
                               slot_off[i] + (mm + 1) * Cs],
                        in_=yqs[0:P])
                    if mm < KT - 1:
                        # one mid step of slot i+1 per stripe boundary
                        # (stripe work hides the eviction latency), then
                        # e0(i+2) units once the chain is emitted
                        if steps:
                            steps.pop(0)()
                        elif units:
                            units.pop(0)()
                            if units:
                                units.pop(0)()
                for step in steps:
                    step()
                for u in units:
                    u()
    nc.compile()
    return nc


def _route_structured(labels):
    """Fit (1024, 1024, M-2048, ceil(maxbig/4)) caps to the histogram.

    Returns (caps, core_clusters[8][2], slot_rows[8][4]) or None if the
    distribution doesn't fit the structured layout."""
    counts = np.bincount(labels, minlength=K)
    if len(counts) != K:
        return None
    order = np.argsort(counts, kind="stable")[::-1]
    b1, b2 = int(order[0]), int(order[1])
    owned = [int(c) for c in order[2:]]
    if len(owned) != 8:
        return None
    M = int(counts[owned].max())
    Mb = int(counts[b1])
    C3 = M - 2048
    C4 = -(-Mb // 4)
    if C3 < 256 or C3 > 2048 or C4 < 256 or C4 > 1024:
        return None
    caps = (1024, 1024, C3, C4)
    if sum(caps) >= 4608:
        return None

    by_cluster = {}
    srt = np.argsort(labels, kind="stable")
    pos = 0
    for c in np.sort(np.unique(labels)):
        cnt = int(counts[c])
        by_cluster[int(c)] = srt[pos:pos + cnt]
        pos += cnt

    core_clusters = []
    slot_rows = []
    for i in range(N_CORES):
        oc = owned[i]
        bc = b1 if i < 4 else b2
        core_clusters.append((oc, bc))
        rows = by_cluster.get(oc, np.empty(0, np.int64))
        srows = [rows[0:1024], rows[1024:2048], rows[2048:]]
        brows = by_cluster.get(bc, np.empty(0, np.int64))
        j = i % 4
        q, r = divmod(len(brows), 4)
        starts = [qq * q + min(qq, r) for qq in range(5)]
        part = brows[starts[j]:starts[j + 1]]
        if len(caps) == 5:
            srows.append(part[:caps[3]])
            srows.append(part[caps[3]:])
        else:
            srows.append(part)
        for s in range(len(caps)):
            if len(srows[s]) > caps[s]:
                return None
        slot_rows.append(srows)
    return caps, core_clusters, slot_rows


def _pack_cluster(wpk, bpk, params, blk, c):
    """Pack cluster c's weights/biases into block blk of wpk/bpk."""
    wb, bb = blk * WSLOT, blk * BSLOT
    we0, we1, we2 = params["w_e0"][c], params["w_e1"][c], params["w_e2"][c]
    wd0, wd1, wd2 = params["w_d0"][c], params["w_d1"][c], params["w_d2"][c]
    for k in range(KT):
        wpk[0:P, wb + _E0 + k * 256: wb + _E0 + (k + 1) * 256] = \
            we0[P * k:P * (k + 1), :]
    for k in range(2):
        wpk[0:128, wb + _E1 + 64 * k: wb + _E1 + 64 * (k + 1)] = \
            we1[128 * k:128 * (k + 1), :]
    wpk[0:64, wb + _E2: wb + _E2 + 16] = we2
    wpk[0:16, wb + _D0: wb + _D0 + 64] = wd0
    wpk[0:64, wb + _D1: wb + _D1 + 256] = wd1
    for k in range(2):
        wpk[0:128, wb + _D2 + 784 * k: wb + _D2 + 784 * (k + 1)] = \
            wd2[128 * k:128 * (k + 1), :]
    be0, be1, be2 = params["b_e0"][c], params["b_e1"][c], params["b_e2"][c]
    bd0, bd1, bd2 = params["b_d0"][c], params["b_d1"][c], params["b_d2"][c]
    bpk[0:128, bb + 0] = be0[0:128]
    bpk[0:128, bb + 1] = be0[128:256]
    bpk[0:64, bb + 2] = be1
    bpk[0:16, bb + 3] = be2
    bpk[0:64, bb + 4] = bd0
    bpk[0:128, bb + 5] = bd1[0:128]
    bpk[0:128, bb + 6] = bd1[128:256]
    for m in range(KT):
        bpk[0:P, bb + 7 + m] = bd2[P * m:P * (m + 1)]


def _flatten_x_structured(xcore_t, caps):
    """[D, sum(caps)] feature-major slab -> chunk-flattened [P, KT*sum]."""
    flat = np.empty((P, KT * sum(caps)), np.float32)
    pos = col = 0
    for C in caps:
        for nch in _slot_chunks(C):
            blk = xcore_t[:, col:col + nch]                   # [784, nch]
            blk = blk.reshape(KT, P, nch).transpose(1, 0, 2)  # [P, KT, nch]
            flat[:, pos:pos + KT * nch] = blk.reshape(P, KT * nch)
            pos += KT * nch
            col += nch
    return flat


def _unflatten_y_structured(yflat, caps):
    """stripe-major [P, KT*sum(caps)] -> row-major [sum(caps), D]."""
    out = np.empty((sum(caps), D), np.float32)
    pos = col = 0
    for C in caps:
        blk = yflat[:, pos:pos + KT * C].reshape(P, KT, C)
        out[col:col + C] = blk.transpose(2, 1, 0).reshape(C, D)
        pos += KT * C
        col += C
    return out


# ---------------------------------------------------------------------------
# fallback path: original fixed-capacity config program (unchanged)
# ---------------------------------------------------------------------------

def _mdt_view(ap, mode):
    return ap.bitcast(_F32R) if mode == "f32r" else ap


def _chunks(R, mode="f32r"):
    if mode == "bf16":
        n = max(1, (R + 511) // 512)
        base, extra = divmod(R, n)
        return [base + (1 if i < extra else 0) for i in range(n)]
    out, rem = [], R
    while rem > 0:
        c = min(512, rem)
        if c == 512 and 0 < rem - c < 256:
            c = max(256, min(512, (rem + 1) // 2))
        out.append(c)
        rem -= c
    return out


def _build_program(S, R, mode):
    mdt = _F32R if mode == "f32r" else _BF16
    idt = _F32 if mode == "f32r" else _BF16
    pipelined = mode == "bf16"
    ncols = S * R
    nflat = ncols * KT
    nc = bacc.Bacc("TRN2", target_bir_lowering=False, debug=False)
    xt = nc.dram_tensor("xt", [P, nflat], idt, kind="ExternalInput").ap()
    wp = nc.dram_tensor("wp", [128, S * WSLOT], idt, kind="ExternalInput").ap()
    bp = nc.dram_tensor("bp", [128, S * BSLOT], _F32, kind="ExternalInput").ap()
    yt = nc.dram_tensor("yt", [P, nflat], idt, kind="ExternalOutput").ap()

    chunks = _chunks(R, mode)
    NCH = len(chunks)
    XQ_BUFS = 3 * NCH if pipelined else NCH + 2
    W_BUFS = 4 if pipelined else 2
    H1_BUFS = 10 if pipelined else 6
    SM_BUFS = 6 if pipelined else 3

    with tile.TileContext(nc) as tc:
        with (
            tc.tile_pool(name="wpool", bufs=1) as wpool,
            tc.tile_pool(name="iopool", bufs=1) as iopool,
            tc.tile_pool(name="apool", bufs=1) as apool,
            tc.tile_pool(name="pspool", bufs=1, space="PSUM") as pspool,
        ):
            bsb = wpool.tile([128, S * BSLOT], _F32, tag="b", name="bsb", bufs=1)
            nc.sync.dma_start(out=bsb, in_=bp)

            wu = wpool.tile([128, 512], _BF16, tag="wu", name="wu", bufs=1)
            nc.vector.memset(wu, 0)
            wups = [pspool.tile([128, 512], _F32, tag="ps", name="wups",
                                bufs=8) for _ in range(4)]
            for i in range(16):
                nc.tensor.matmul(wups[i % 4], wu[:, 0:128], wu,
                                 start=True, stop=True)

            def bias(lo, col):
                return bsb[0:lo, col:col + 1]

            def ps_tile(parts, nch):
                return pspool.tile([parts, nch], _F32, tag="ps", name="ps",
                                   bufs=8)

            drain_i = [0]

            def drain_relu(out, ps, bias_ap):
                drain_i[0] += 1
                if drain_i[0] % 2:
                    nc.scalar.activation(out, ps, _RELU, bias=bias_ap)
                else:
                    nc.vector.tensor_scalar(out, ps, bias_ap, 0.0,
                                            mybir.AluOpType.add,
                                            mybir.AluOpType.max)

            def drain_bias(out, ps, bias_ap):
                drain_i[0] += 1
                if drain_i[0] % 2:
                    nc.scalar.add(out, ps, bias_ap)
                else:
                    nc.vector.tensor_scalar_add(out, ps, bias_ap)

            res = {}

            def ensure_slot(s):
                if s in res or s >= S:
                    return
                w = wpool.tile([128, WSLOT], mdt, tag="w", name="w",
                               bufs=W_BUFS)
                nc.sync.dma_start(
                    out=w[:, 0:_E1],
                    in_=_mdt_view(wp[:, s * WSLOT:s * WSLOT + _E1], mode))
                nc.sync.dma_start(
                    out=w[:, _E1:],
                    in_=_mdt_view(wp[:, s * WSLOT + _E1:(s + 1) * WSLOT], mode))
                offs = []
                cum = s * R * KT
                for nch in chunks:
                    offs.append(cum)
                    cum += nch * KT
                xq = []
                for ci, nch in enumerate(chunks):
                    t = iopool.tile([128, KT, nch], mdt, tag="xq", name="xq",
                                    bufs=XQ_BUFS)
                    nc.sync.dma_start(
                        out=t[0:P],
                        in_=_mdt_view(
                            xt[:, offs[ci]:offs[ci] + KT * nch]
                            .rearrange("p (k n) -> p k n", k=KT), mode))
                    xq.append(t)
                res[s] = {"w": w, "xq": xq, "offs": offs, "bb": s * BSLOT,
                          "h1": [[None] * NCH, [None] * NCH],
                          "e0ps": [None, None]}

            def e0_group(s, m, k):
                r = res[s]
                if k == 0:
                    r["e0ps"][m] = [ps_tile(128, nch) for nch in chunks]
                wk = r["w"][0:P, _E0 + k * 256 + 128 * m:
                            _E0 + k * 256 + 128 * m + 128]
                for ci, nch in enumerate(chunks):
                    nc.tensor.matmul(r["e0ps"][m][ci], wk,
                                     r["xq"][ci][0:P, k, :],
                                     start=(k == 0), stop=(k == KT - 1))
                if k == KT - 1:
                    for ci, nch in enumerate(chunks):
                        t = apool.tile([128, nch], mdt, tag="h1", name="h1",
                                       bufs=H1_BUFS)
                        drain_relu(t, r["e0ps"][m][ci], bias(128, r["bb"] + m))
                        r["h1"][m][ci] = t
                    r["e0ps"][m] = None

            E0_ORDER = [(m, k) for m in range(2) for k in range(KT)]

            if pipelined:
                ensure_slot(0)
                ensure_slot(1)
                r0 = res[0]
                for ci, nch in enumerate(chunks):
                    for m in range(2):
                        ps0 = ps_tile(128, nch)
                        for k in range(KT):
                            wk = r0["w"][0:P, _E0 + k * 256 + 128 * m:
                                         _E0 + k * 256 + 128 * m + 128]
                            nc.tensor.matmul(ps0, wk, r0["xq"][ci][0:P, k, :],
                                             start=(k == 0), stop=(k == KT - 1))
                        t = apool.tile([128, nch], mdt, tag="h1", name="h1",
                                       bufs=H1_BUFS)
                        drain_relu(t, ps0, bias(128, r0["bb"] + m))
                        r0["h1"][m][ci] = t

            for s in range(S):
                if pipelined:
                    ensure_slot(s + 2)
                    filler = iter(E0_ORDER) if s + 1 < S else iter([])
                else:
                    ensure_slot(s)
                    for m, k in E0_ORDER:
                        e0_group(s, m, k)
                    filler = iter([])

                def fill(n):
                    for _ in range(n):
                        mk = next(filler, None)
                        if mk is not None:
                            e0_group(s + 1, *mk)

                r = res[s]
                w, bb, offs, h1 = r["w"], r["bb"], r["offs"], r["h1"]

                ps = [None] * NCH
                for k in range(2):
                    wk = w[0:128, _E1 + 64 * k:_E1 + 64 * k + 64]
                    for ci, nch in enumerate(chunks):
                        if k == 0:
                            ps[ci] = ps_tile(64, nch)
                        nc.tensor.matmul(ps[ci], wk, h1[k][ci],
                                         start=(k == 0), stop=(k == 1))
                h2 = []
                for ci, nch in enumerate(chunks):
                    t = apool.tile([64, nch], mdt, tag="h2", name="h2", bufs=SM_BUFS)
                    drain_relu(t, ps[ci], bias(64, bb + 2))
                    h2.append(t)
                fill(2)

                ps = [None] * NCH
                wk = w[0:64, _E2:_E2 + 16]
                for ci, nch in enumerate(chunks):
                    ps[ci] = ps_tile(16, nch)
                    nc.tensor.matmul(ps[ci], wk, h2[ci], start=True, stop=True)
                z = []
                for ci, nch in enumerate(chunks):
                    t = apool.tile([16, nch], mdt, tag="z", name="z", bufs=SM_BUFS)
                    drain_relu(t, ps[ci], bias(16, bb + 3))
                    z.append(t)
                fill(2)

                ps = [None] * NCH
                wk = w[0:16, _D0:_D0 + 64]
                for ci, nch in enumerate(chunks):
                    ps[ci] = ps_tile(64, nch)
                    nc.tensor.matmul(ps[ci], wk, z[ci], start=True, stop=True)
                a1 = []
                for ci, nch in enumerate(chunks):
                    t = apool.tile([64, nch], mdt, tag="a1", name="a1", bufs=SM_BUFS)
                    drain_relu(t, ps[ci], bias(64, bb + 4))
                    a1.append(t)
                fill(2)

                a2 = [[None] * NCH, [None] * NCH]
                for m in range(2):
                    wk = w[0:64, _D1 + 128 * m:_D1 + 128 * m + 128]
                    ps = [None] * NCH
                    for ci, nch in enumerate(chunks):
                        ps[ci] = ps_tile(128, nch)
                        nc.tensor.matmul(ps[ci], wk, a1[ci],
                                         start=True, stop=True)
                    for ci, nch in enumerate(chunks):
                        t = apool.tile([128, nch], mdt, tag="a2", name="a2",
                                       bufs=7)
                        drain_relu(t, ps[ci], bias(128, bb + 5 + m))
                        a2[m][ci] = t
                    fill(2)

                yq = []
                for ci, nch in enumerate(chunks):
                    yq.append(iopool.tile([128, KT, nch], idt, tag="yq",
                                          name="yq", bufs=NCH + (3 if pipelined else 1)))
                for mm in range(KT):
                    ps = [None] * NCH
                    for k in range(2):
                        wk = w[0:128, _D2 + 784 * k + 112 * mm:
                               _D2 + 784 * k + 112 * mm + 112]
                        for ci, nch in enumerate(chunks):
                            if k == 0:
                                ps[ci] = ps_tile(112, nch)
                            nc.tensor.matmul(ps[ci], wk, a2[k][ci],
                                             start=(k == 0), stop=(k == 1))
                    for ci, nch in enumerate(chunks):
                        drain_bias(yq[ci][0:P, mm, :], ps[ci],
                                   bias(112, bb + 7 + mm))
                    if mm < 4:
                        fill(1)
                fill(14)
                for ci, nch in enumerate(chunks):
                    nc.sync.dma_start(
                        out=yt[:, offs[ci]:offs[ci] + KT * nch]
                        .rearrange("p (k n) -> p k n", k=KT),
                        in_=yq[ci][0:P])
                del res[s]
    nc.compile()
    return nc


_programs = {}


def _get_program(key, builder):
    if key not in _programs:
        _programs[key] = builder()
    return _programs[key]


def _pack_weights(params, slot_clusters):
    S = len(slot_clusters)
    wpk = np.zeros((128, S * WSLOT), np.float32)
    bpk = np.zeros((128, S * BSLOT), np.float32)
    for s, c in enumerate(slot_clusters):
        _pack_cluster(wpk, bpk, params, s, c)
    return wpk, bpk


def _route(labels, mode):
    counts = np.bincount(labels, minlength=K)
    configs = _CONFIGS if mode == "bf16" else _CONFIGS[1:]
    for S, R in configs:
        need = int(np.sum((counts + R - 1) // R))
        if need <= N_CORES * S:
            break
    nslots = N_CORES * S
    order = np.argsort(labels, kind="stable")
    slot_cluster = np.zeros(nslots, np.int64)
    slot_rows = [np.empty(0, np.int64)] * nslots
    si = pos = 0
    for c in range(K):
        cnt = int(counts[c])
        rows_c = order[pos:pos + cnt]
        pos += cnt
        for off in range(0, cnt, R):
            slot_cluster[si] = c
            slot_rows[si] = rows_c[off:off + R]
            si += 1
    return S, R, slot_cluster, slot_rows


def _flatten_xcore(xcore_t, R, chunks):
    ncols = xcore_t.shape[1]
    S = ncols // R
    flat = np.empty((P, ncols * KT), np.float32)
    pos = 0
    for s in range(S):
        col = s * R
        for nch in chunks:
            blk = xcore_t[:, col:col + nch]
            blk = blk.reshape(KT, P, nch).transpose(1, 0, 2)
            flat[:, pos:pos + KT * nch] = blk.reshape(P, KT * nch)
            pos += KT * nch
            col += nch
    return flat


def _unflatten_ycore(yflat, R, chunks):
    ncols = yflat.shape[1] // KT
    S = ncols // R
    out = np.empty((ncols, D), np.float32)
    pos = 0
    for s in range(S):
        col = s * R
        for nch in chunks:
            blk = yflat[:, pos:pos + KT * nch].reshape(P, KT, nch)
            out[col:col + nch] = blk.transpose(2, 1, 0).reshape(nch, D)
            pos += KT * nch
            col += nch
    return out


def _run_structured(x, params, strat, trace):
    import ml_dtypes
    caps, core_clusters, slot_rows = strat
    nc = _get_program(("st",) + tuple(caps),
                      lambda: _build_program_structured(caps))
    ncols = sum(caps)
    in_maps = []
    for i in range(N_CORES):
        xcore = np.zeros((ncols, D), np.float32)
        col = 0
        for s in range(len(caps)):
            rows = slot_rows[i][s]
            if len(rows):
                xcore[col:col + len(rows)] = x[rows]
            col += caps[s]
        wpk = np.zeros((128, 2 * WSLOT), np.float32)
        bpk = np.zeros((128, 2 * BSLOT), np.float32)
        _pack_cluster(wpk, bpk, params, 0, core_clusters[i][0])
        _pack_cluster(wpk, bpk, params, 1, core_clusters[i][1])
        xflat = _flatten_x_structured(np.ascontiguousarray(xcore.T), caps)
        in_maps.append({"xt": xflat.astype(ml_dtypes.bfloat16),
                        "wp": wpk.astype(ml_dtypes.bfloat16),
                        "bp": bpk})
    res = run_bass_kernel_spmd(nc, in_maps, core_ids=list(range(N_CORES)),
                               trace=trace)
    out = np.zeros_like(x)
    for i in range(N_CORES):
        yraw = np.asarray(res.results[i]["yt"]).astype(np.float32)
        ytT = _unflatten_y_structured(yraw, caps)
        col = 0
        for s in range(len(caps)):
            rows = slot_rows[i][s]
            if len(rows):
                out[rows] = ytT[col:col + len(rows)]
            col += caps[s]
    return out, res


def _run_generic(x, params, labels, mode, trace):
    S, R, slot_cluster, slot_rows = _route(labels, mode)
    chunks = _chunks(R, mode)
    nc = _get_program((S, R, mode), lambda: _build_program(S, R, mode))
    in_maps = []
    for i in range(N_CORES):
        xcore = np.zeros((S * R, D), np.float32)
        for s in range(S):
            rows = slot_rows[i * S + s]
            if len(rows):
                xcore[s * R: s * R + len(rows)] = x[rows]
        wpk, bpk = _pack_weights(params, slot_cluster[i * S:(i + 1) * S])
        xflat = _flatten_xcore(np.ascontiguousarray(xcore.T), R, chunks)
        if mode == "bf16":
            import ml_dtypes
            xflat = xflat.astype(ml_dtypes.bfloat16)
            wpk = wpk.astype(ml_dtypes.bfloat16)
        in_maps.append({"xt": xflat, "wp": wpk, "bp": bpk})
    res = run_bass_kernel_spmd(nc, in_maps, core_ids=list(range(N_CORES)),
                               trace=trace)
    out = np.zeros_like(x)
    for i in range(N_CORES):
        yraw = np.asarray(res.results[i]["yt"]).astype(np.float32)
        ytT = _unflatten_ycore(yraw, R, chunks)
        for s in range(S):
            rows = slot_rows[i * S + s]
            if len(rows):
                out[rows] = ytT[s * R: s * R + len(rows)]
    return out, res


def kernel_traced(inputs, trace=False, mode=None):
    if mode is None:
        mode = MODE
    x = np.ascontiguousarray(np.asarray(inputs["x"], dtype=np.float32))
    labels = np.asarray(inputs["kmeans_label"]).astype(np.int64).ravel()
    params = {k: np.asarray(v, dtype=np.float32)
              for k, v in inputs.items() if k not in ("x", "kmeans_label")}

    if mode == "bf16":
        strat = _route_structured(labels)
        if strat is not None:
            return _run_structured(x, params, strat, trace)
    return _run_generic(x, params, labels, mode, trace)


def kernel(**inputs):
    out, _ = kernel_traced(inputs, trace=False)
    return out


# revision 13
# speedup vs baseline: 1.0493x; 1.0493x over previous
"""MoE-routed K-cluster autoencoder kernel for 8 Trainium2 NeuronCores.

Strategy
--------
Each row of x is reconstructed by the autoencoder of its kmeans cluster.
Computing all K experts densely for every row (like the reference) does
10x the needed matmul work, so we *route*.

Structured path (default for ~uniform labels): the label histogram is
known at call time, so slot capacities are fitted to it.  The two
largest clusters are split 4 ways into the per-core "col4" slot (4 cores
each); the remaining 8 clusters are each *owned* by one core and span
that core's slots 0-2, which share a single weight load.  Per-core slot
capacities (1024, 1024, M-2048, ceil(maxbig/4)) give ~4170 row-slots vs
4608 for the old fixed-capacity config (-10% PE work and x/y bytes), and
owning one cluster per core halves the weight DMA (2 unique clusters per
core instead of 4).  Chunks are ~512 columns (one PSUM bank) to minimize
LDWEIGHTS re-issues and PSUM->SBUF eviction instruction count.  The
decoder output is drained stripe-major and each 112-feature stripe is
DMA'd out as soon as it is evicted, so the final slot exposes only one
stripe's DMA instead of the whole slot.  Slot 0's first x chunk lands in
two k-split DMAs so e0 can start after ~half the chunk arrives.

Fallback path (skewed/degenerate labels): the original fixed-capacity
slot config search, unchanged.

Device pipeline (both paths): per slot, the 6-layer MLP chain runs as
feature-major matmuls (outT = W.T @ actsT); the next slot's encoder-0
work is software-pipelined into the current slot's serial mid-layers;
PSUM->SBUF bias+ReLU evictions alternate between ScalarE and VectorE; a
short pre-warm matmul burst opens the HAM clock gate while the first
DMAs land.  bf16 operands end-to-end (~5.6e-3 scale-relative error).
"""

import numpy as np

import concourse.tile as tile
from concourse import bacc, mybir
from concourse.bass_utils import run_bass_kernel_spmd

N_CORES = 8
B, D, H1, H2, L, K = 32768, 784, 256, 64, 16, 10
P = 112          # partition tile height for the D axis: 784 = 7 * 112
KT = D // P      # 7 k-tiles along D

# packed weight layout (column offsets in a [128, WSLOT] block)
_E0, _E1, _E2, _D0, _D1, _D2 = 0, 1792, 1920, 1936, 2000, 2256
WSLOT = 3824     # = 7*256 + 2*64 + 16 + 64 + 256 + 2*784
BSLOT = 14       # bias columns per block: 2 + 1 + 1 + 1 + 2 + 7

# (slots_per_core, rows_per_slot) fallback configs
_CONFIGS = [(4, 1152), (4, 1280), (8, 640), (16, 320), (32, 160)]

_F32 = mybir.dt.float32
_F32R = mybir.dt.float32r
_BF16 = mybir.dt.bfloat16
_RELU = mybir.ActivationFunctionType.Relu

MODE = "bf16"


def _slot_chunks(C):
    """Split C columns into equal-ish chunks of <=512 (one PSUM bank)."""
    n = max(1, (C + 511) // 512)
    base, extra = divmod(C, n)
    return [base + (1 if i < extra else 0) for i in range(n)]


# ---------------------------------------------------------------------------
# structured program: caps per slot, slots 0-2 share weight block 0,
# slot 3 uses weight block 1.  x chunk-flattened; y stripe-major.
# ---------------------------------------------------------------------------

def _build_program_structured(caps):
    S = len(caps)
    chunk_lists = [_slot_chunks(c) for c in caps]
    XQ_BUFS = sum(len(cl) for cl in chunk_lists)
    nflat = KT * sum(caps)
    nc = bacc.Bacc("TRN2", target_bir_lowering=False, debug=False)
    xt = nc.dram_tensor("xt", [P, nflat], _BF16, kind="ExternalInput").ap()
    wp = nc.dram_tensor("wp", [128, 2 * WSLOT], _BF16, kind="ExternalInput").ap()
    bp = nc.dram_tensor("bp", [128, 2 * BSLOT], _F32, kind="ExternalInput").ap()
    yt = nc.dram_tensor("yt", [P, nflat], _BF16, kind="ExternalOutput").ap()

    # per-slot x column offsets (in xt/yt, units of columns)
    slot_off = []
    cum = 0
    for c in caps:
        slot_off.append(cum)
        cum += KT * c

    # slot -> weight/bias block: slots 0-2 share the owned cluster's
    # weights; the remaining slot(s) hold the big cluster's
    wblk = [0, 0, 0] + [1] * (S - 3)

    with tile.TileContext(nc) as tc:
        with (
            tc.tile_pool(name="wpool", bufs=1) as wpool,
            tc.tile_pool(name="iopool", bufs=1) as iopool,
            tc.tile_pool(name="apool", bufs=1) as apool,
            tc.tile_pool(name="pspool", bufs=1, space="PSUM") as pspool,
        ):
            bsb = wpool.tile([128, 2 * BSLOT], _F32, tag="b", name="bsb", bufs=1)
            wA = wpool.tile([128, WSLOT], _BF16, tag="wA", name="wA", bufs=1)
            wB = wpool.tile([128, WSLOT], _BF16, tag="wB", name="wB", bufs=1)

            xq = {}  # (s, ci) -> tile

            def fetch_slot_x(s, eng, ksplit=False):
                if s >= S:
                    return
                off = slot_off[s]
                for ci, nch in enumerate(chunk_lists[s]):
                    t = iopool.tile([128, KT, nch], _BF16, tag="xq",
                                    name="xq", bufs=XQ_BUFS)
                    src = xt[:, off:off + KT * nch].rearrange(
                        "p (k n) -> p k n", k=KT)
                    if ksplit and ci == 0:
                        # subtile deps let e0's k<4 matmuls start after
                        # the first half of the chunk lands
                        eng.dma_start(out=t[0:P, 0:4], in_=src[:, 0:4])
                        eng.dma_start(out=t[0:P, 4:KT], in_=src[:, 4:KT])
                    else:
                        eng.dma_start(out=t[0:P], in_=src)
                    xq[(s, ci)] = t
                    off += KT * nch

            # The kernel runs close to the per-core DMA throughput
            # ceiling, so ALL x tiles are fetched up front (they fit in
            # SBUF) -- the queues stay saturated from t=0 and the e0
            # units never wait on a just-in-time prefetch.  Enqueues
            # (~0.6us each, serialized per engine) are split across the
            # two HWDGE engines so they don't gate the first e0.
            fetch_slot_x(0, nc.sync, ksplit=True)
            nc.scalar.dma_start(out=wA[:, 0:_E1], in_=wp[:, 0:_E1])
            nc.sync.dma_start(out=bsb, in_=bp)
            nc.scalar.dma_start(out=wA[:, _E1:], in_=wp[:, _E1:WSLOT])
            fetch_slot_x(1, nc.sync)
            nc.scalar.dma_start(out=wB[:, 0:_E1],
                                in_=wp[:, WSLOT:WSLOT + _E1])
            fetch_slot_x(2, nc.sync)
            nc.scalar.dma_start(out=wB[:, _E1:],
                                in_=wp[:, WSLOT + _E1:2 * WSLOT])
            fetch_slot_x(3, nc.scalar)
            for s5 in range(4, S):
                fetch_slot_x(s5, nc.sync)

            # PE pre-warm: a few throwaway matmuls get PE activity started
            # (HAM clock-gate) while the first DMAs land, without delaying
            # e0 behind a long in-order warmup burst.
            wu = wpool.tile([128, 512], _BF16, tag="wu", name="wu", bufs=1)
            nc.gpsimd.memset(wu, 0)
            wups = [pspool.tile([128, 512], _F32, tag="ps", name="wups",
                                bufs=8) for _ in range(4)]
            for i in range(6):
                nc.tensor.matmul(wups[i % 4], wu[:, 0:128], wu,
                                 start=True, stop=True)

            def wt(s):
                return wA if wblk[s] == 0 else wB

            def bias(s, lo, col):
                bb = wblk[s] * BSLOT
                return bsb[0:lo, bb + col:bb + col + 1]

            def ps_tile(parts, nch):
                return pspool.tile([parts, nch], _F32, tag="ps", name="ps",
                                   bufs=8)

            drain_i = [0]

            def drain_relu(out, ps, bias_ap):
                drain_i[0] += 1
                if drain_i[0] % 2:
                    nc.scalar.activation(out, ps, _RELU, bias=bias_ap)
                else:
                    nc.vector.tensor_scalar(out, ps, bias_ap, 0.0,
                                            mybir.AluOpType.add,
                                            mybir.AluOpType.max)

            def drain_bias(out, ps, bias_ap):
                drain_i[0] += 1
                if drain_i[0] % 2:
                    nc.scalar.add(out, ps, bias_ap)
                else:
                    nc.vector.tensor_scalar_add(out, ps, bias_ap)

            h1s = {}   # s -> [m][ci] h1 tiles
            a2s = {}   # s -> [m][ci] a2 tiles

            def e0_unit(s, ci, m):
                """One e0 unit: chunk ci, m-half m, all 7 k accumulations."""
                nch = chunk_lists[s][ci]
                w = wt(s)
                ps = ps_tile(128, nch)
                for k in range(KT):
                    wk = w[0:P, _E0 + k * 256 + 128 * m:
                           _E0 + k * 256 + 128 * m + 128]
                    nc.tensor.matmul(ps, wk, xq[(s, ci)][0:P, k, :],
                                     start=(k == 0), stop=(k == KT - 1))
                t = apool.tile([128, nch], _BF16, tag="h1", name="h1",
                               bufs=14)
                drain_relu(t, ps, bias(s, 128, m))
                h1s.setdefault(s, [{}, {}])[m][ci] = t

            def mid_steps(s):
                """The serial e1->e2->d0->d1 chain of slot s as 5 steps.

                Consecutive steps must be separated by ~1us of unrelated
                PE work (a d2 stripe of the previous slot) to hide the
                PSUM-eviction latency between layers."""
                if s >= S:
                    return
                chunks = chunk_lists[s]
                NCH = len(chunks)
                w = wt(s)
                st = {}

                def e1():
                    h1 = h1s[s]
                    ps = [None] * NCH
                    for k in range(2):
                        wk = w[0:128, _E1 + 64 * k:_E1 + 64 * k + 64]
                        for ci, nch in enumerate(chunks):
                            if k == 0:
                                ps[ci] = ps_tile(64, nch)
                            nc.tensor.matmul(ps[ci], wk, h1[k][ci],
                                             start=(k == 0), stop=(k == 1))
                    st["h2"] = []
                    for ci, nch in enumerate(chunks):
                        t = apool.tile([64, nch], _BF16, tag="h2", name="h2",
                                       bufs=4)
                        drain_relu(t, ps[ci], bias(s, 64, 2))
                        st["h2"].append(t)

                def e2():
                    ps = [None] * NCH
                    wk = w[0:64, _E2:_E2 + 16]
                    for ci, nch in enumerate(chunks):
                        ps[ci] = ps_tile(16, nch)
                        nc.tensor.matmul(ps[ci], wk, st["h2"][ci],
                                         start=True, stop=True)
                    st["z"] = []
                    for ci, nch in enumerate(chunks):
                        t = apool.tile([16, nch], _BF16, tag="z", name="z",
                                       bufs=4)
                        drain_relu(t, ps[ci], bias(s, 16, 3))
                        st["z"].append(t)

                def d0():
                    ps = [None] * NCH
                    wk = w[0:16, _D0:_D0 + 64]
                    for ci, nch in enumerate(chunks):
                        ps[ci] = ps_tile(64, nch)
                        nc.tensor.matmul(ps[ci], wk, st["z"][ci],
                                         start=True, stop=True)
                    st["a1"] = []
                    for ci, nch in enumerate(chunks):
                        t = apool.tile([64, nch], _BF16, tag="a1", name="a1",
                                       bufs=4)
                        drain_relu(t, ps[ci], bias(s, 64, 4))
                        st["a1"].append(t)

                def d1(m):
                    a2 = a2s.setdefault(s, [[None] * NCH, [None] * NCH])
                    wk = w[0:64, _D1 + 128 * m:_D1 + 128 * m + 128]
                    ps = [None] * NCH
                    for ci, nch in enumerate(chunks):
                        ps[ci] = ps_tile(128, nch)
                        nc.tensor.matmul(ps[ci], wk, st["a1"][ci],
                                         start=True, stop=True)
                    for ci, nch in enumerate(chunks):
                        t = apool.tile([128, nch], _BF16, tag="a2", name="a2",
                                       bufs=13)
                        drain_relu(t, ps[ci], bias(s, 128, 5 + m))
                        a2[m][ci] = t

                yield e1
                yield e2
                yield d0
                yield lambda: d1(0)
                yield lambda: d1(1)

            def e0_unit_steps(s):
                if s >= S:
                    return
                for ci in range(len(chunk_lists[s])):
                    for m in range(2):
                        yield (lambda ci=ci, m=m: e0_unit(s, ci, m))

            # ---- prologue: e0(0) inline; mid(0) interleaved with e0(1)
            for ci in range(len(chunk_lists[0])):
                for m in range(2):
                    e0_unit(0, ci, m)
            u1 = e0_unit_steps(1)
            for step in mid_steps(0):
                step()
                u = next(u1, None)
                if u is not None:
                    u()
            for u in u1:
                u()

            # ---- main pipeline: iteration i runs d2(i) stripes with
            # mid(i+1) steps and e0(i+2) units interleaved between stripes
            for i in range(S):
                chunks = chunk_lists[i]
                NCH = len(chunks)
                w = wt(i)
                steps = list(mid_steps(i + 1))
                units = list(e0_unit_steps(i + 2))
                a2 = a2s[i]
                col_off = []
                c0 = 0
                for nch in chunks:
                    col_off.append(c0)
                    c0 += nch
                Cs = caps[i]
                for mm in range(KT):
                    yqs = iopool.tile([112, Cs], _BF16, tag="yq", name="yqs",
                                      bufs=8)
                    ps = [None] * NCH
                    for k in range(2):
                        wk = w[0:128, _D2 + 784 * k + 112 * mm:
                               _D2 + 784 * k + 112 * mm + 112]
                        for ci, nch in enumerate(chunks):
                            if k == 0:
                                ps[ci] = ps_tile(112, nch)
                            nc.tensor.matmul(ps[ci], wk, a2[k][ci],
                                             start=(k == 0), stop=(k == 1))
                    for ci, nch in enumerate(chunks):
                        drain_bias(yqs[0:P, col_off[ci]:col_off[ci] + nch],
                                   ps[ci], bias(i, 112, 7 + mm))
                    nc.sync.dma_start(
                        out=yt[:, slot_off[i] + mm * Cs:
                               slot_off[i] + (mm + 1) * Cs],
                        in_=yqs[0:P])
                    if mm < KT - 1:
                        # one mid step of slot i+1 per stripe boundary
                        # (stripe work hides the eviction latency), then
                        # e0(i+2) units once the chain is emitted
                        if steps:
                            steps.pop(0)()
                        elif units:
                            units.pop(0)()
                            if units:
                                units.pop(0)()
                for step in steps:
                    step()
                for u in units:
                    u()
    nc.compile()
    return nc


def _route_structured(labels):
    """Fit (1024, 1024, M-2048, ceil(maxbig/4)) caps to the histogram.

    Returns (caps, core_clusters[8][2], slot_rows[8][4]) or None if the
    distribution doesn't fit the structured layout."""
    counts = np.bincount(labels, minlength=K)
    if len(counts) != K:
        return None
    order = np.argsort(counts, kind="stable")[::-1]
    b1, b2 = int(order[0]), int(order[1])
    owned = [int(c) for c in order[2:]]
    if len(owned) != 8:
        return None
    M = int(counts[owned].max())
    Mb = int(counts[b1])
    C3 = M - 2048
    C4 = -(-Mb // 4)
    if C3 < 256 or C3 > 2048 or C4 < 256 or C4 > 1024:
        return None
    caps = (1024, 1024, C3, C4)
    if sum(caps) >= 4608:
        return None

    by_cluster = {}
    srt = np.argsort(labels, kind="stable")
    pos = 0
    for c in np.sort(np.unique(labels)):
        cnt = int(counts[c])
        by_cluster[int(c)] = srt[pos:pos + cnt]
        pos += cnt

    core_clusters = []
    slot_rows = []
    for i in range(N_CORES):
        oc = owned[i]
        bc = b1 if i < 4 else b2
        core_clusters.append((oc, bc))
        rows = by_cluster.get(oc, np.empty(0, np.int64))
        srows = [rows[0:1024], rows[1024:2048], rows[2048:]]
        brows = by_cluster.get(bc, np.empty(0, np.int64))
        j = i % 4
        q, r = divmod(len(brows), 4)
        starts = [qq * q + min(qq, r) for qq in range(5)]
        part = brows[starts[j]:starts[j + 1]]
        if len(caps) == 5:
            srows.append(part[:caps[3]])
            srows.append(part[caps[3]:])
        else:
            srows.append(part)
        for s in range(len(caps)):
            if len(srows[s]) > caps[s]:
                return None
        slot_rows.append(srows)
    return caps, core_clusters, slot_rows


def _pack_cluster(wpk, bpk, params, blk, c):
    """Pack cluster c's weights/biases into block blk of wpk/bpk."""
    wb, bb = blk * WSLOT, blk * BSLOT
    we0, we1, we2 = params["w_e0"][c], params["w_e1"][c], params["w_e2"][c]
    wd0, wd1, wd2 = params["w_d0"][c], params["w_d1"][c], params["w_d2"][c]
    for k in range(KT):
        wpk[0:P, wb + _E0 + k * 256: wb + _E0 + (k + 1) * 256] = \
            we0[P * k:P * (k + 1), :]
    for k in range(2):
        wpk[0:128, wb + _E1 + 64 * k: wb + _E1 + 64 * (k + 1)] = \
            we1[128 * k:128 * (k + 1), :]
    wpk[0:64, wb + _E2: wb + _E2 + 16] = we2
    wpk[0:16, wb + _D0: wb + _D0 + 64] = wd0
    wpk[0:64, wb + _D1: wb + _D1 + 256] = wd1
    for k in range(2):
        wpk[0:128, wb + _D2 + 784 * k: wb + _D2 + 784 * (k + 1)] = \
            wd2[128 * k:128 * (k + 1), :]
    be0, be1, be2 = params["b_e0"][c], params["b_e1"][c], params["b_e2"][c]
    bd0, bd1, bd2 = params["b_d0"][c], params["b_d1"][c], params["b_d2"][c]
    bpk[0:128, bb + 0] = be0[0:128]
    bpk[0:128, bb + 1] = be0[128:256]
    bpk[0:64, bb + 2] = be1
    bpk[0:16, bb + 3] = be2
    bpk[0:64, bb + 4] = bd0
    bpk[0:128, bb + 5] = bd1[0:128]
    bpk[0:128, bb + 6] = bd1[128:256]
    for m in range(KT):
        bpk[0:P, bb + 7 + m] = bd2[P * m:P * (m + 1)]


def _flatten_x_structured(xcore_t, caps):
    """[D, sum(caps)] feature-major slab -> chunk-flattened [P, KT*sum]."""
    flat = np.empty((P, KT * sum(caps)), np.float32)
    pos = col = 0
    for C in caps:
        for nch in _slot_chunks(C):
            blk = xcore_t[:, col:col + nch]                   # [784, nch]
            blk = blk.reshape(KT, P, nch).transpose(1, 0, 2)  # [P, KT, nch]
            flat[:, pos:pos + KT * nch] = blk.reshape(P, KT * nch)
            pos += KT * nch
            col += nch
    return flat


def _unflatten_y_structured(yflat, caps):
    """stripe-major [P, KT*sum(caps)] -> row-major [sum(caps), D]."""
    out = np.empty((sum(caps), D), np.float32)
    pos = col = 0
    for C in caps:
        blk = yflat[:, pos:pos + KT * C].reshape(P, KT, C)
        out[col:col + C] = blk.transpose(2, 1, 0).reshape(C, D)
        pos += KT * C
        col += C
    return out


# ---------------------------------------------------------------------------
# fallback path: original fixed-capacity config program (unchanged)
# ---------------------------------------------------------------------------

def _mdt_view(ap, mode):
    return ap.bitcast(_F32R) if mode == "f32r" else ap


def _chunks(R, mode="f32r"):
    if mode == "bf16":
        n = max(1, (R + 511) // 512)
        base, extra = divmod(R, n)
        return [base + (1 if i < extra else 0) for i in range(n)]
    out, rem = [], R
    while rem > 0:
        c = min(512, rem)
        if c == 512 and 0 < rem - c < 256:
            c = max(256, min(512, (rem + 1) // 2))
        out.append(c)
        rem -= c
    return out


def _build_program(S, R, mode):
    mdt = _F32R if mode == "f32r" else _BF16
    idt = _F32 if mode == "f32r" else _BF16
    pipelined = mode == "bf16"
    ncols = S * R
    nflat = ncols * KT
    nc = bacc.Bacc("TRN2", target_bir_lowering=False, debug=False)
    xt = nc.dram_tensor("xt", [P, nflat], idt, kind="ExternalInput").ap()
    wp = nc.dram_tensor("wp", [128, S * WSLOT], idt, kind="ExternalInput").ap()
    bp = nc.dram_tensor("bp", [128, S * BSLOT], _F32, kind="ExternalInput").ap()
    yt = nc.dram_tensor("yt", [P, nflat], idt, kind="ExternalOutput").ap()

    chunks = _chunks(R, mode)
    NCH = len(chunks)
    XQ_BUFS = 3 * NCH if pipelined else NCH + 2
    W_BUFS = 4 if pipelined else 2
    H1_BUFS = 10 if pipelined else 6
    SM_BUFS = 6 if pipelined else 3

    with tile.TileContext(nc) as tc:
        with (
            tc.tile_pool(name="wpool", bufs=1) as wpool,
            tc.tile_pool(name="iopool", bufs=1) as iopool,
            tc.tile_pool(name="apool", bufs=1) as apool,
            tc.tile_pool(name="pspool", bufs=1, space="PSUM") as pspool,
        ):
            bsb = wpool.tile([128, S * BSLOT], _F32, tag="b", name="bsb", bufs=1)
            nc.sync.dma_start(out=bsb, in_=bp)

            wu = wpool.tile([128, 512], _BF16, tag="wu", name="wu", bufs=1)
            nc.vector.memset(wu, 0)
            wups = [pspool.tile([128, 512], _F32, tag="ps", name="wups",
                                bufs=8) for _ in range(4)]
            for i in range(16):
                nc.tensor.matmul(wups[i % 4], wu[:, 0:128], wu,
                                 start=True, stop=True)

            def bias(lo, col):
                return bsb[0:lo, col:col + 1]

            def ps_tile(parts, nch):
                return pspool.tile([parts, nch], _F32, tag="ps", name="ps",
                                   bufs=8)

            drain_i = [0]

            def drain_relu(out, ps, bias_ap):
                drain_i[0] += 1
                if drain_i[0] % 2:
                    nc.scalar.activation(out, ps, _RELU, bias=bias_ap)
                else:
                    nc.vector.tensor_scalar(out, ps, bias_ap, 0.0,
                                            mybir.AluOpType.add,
                                            mybir.AluOpType.max)

            def drain_bias(out, ps, bias_ap):
                drain_i[0] += 1
                if drain_i[0] % 2:
                    nc.scalar.add(out, ps, bias_ap)
                else:
                    nc.vector.tensor_scalar_add(out, ps, bias_ap)

            res = {}

            def ensure_slot(s):
                if s in res or s >= S:
                    return
                w = wpool.tile([128, WSLOT], mdt, tag="w", name="w",
                               bufs=W_BUFS)
                nc.sync.dma_start(
                    out=w[:, 0:_E1],
                    in_=_mdt_view(wp[:, s * WSLOT:s * WSLOT + _E1], mode))
                nc.sync.dma_start(
                    out=w[:, _E1:],
                    in_=_mdt_view(wp[:, s * WSLOT + _E1:(s + 1) * WSLOT], mode))
                offs = []
                cum = s * R * KT
                for nch in chunks:
                    offs.append(cum)
                    cum += nch * KT
                xq = []
                for ci, nch in enumerate(chunks):
                    t = iopool.tile([128, KT, nch], mdt, tag="xq", name="xq",
                                    bufs=XQ_BUFS)
                    nc.sync.dma_start(
                        out=t[0:P],
                        in_=_mdt_view(
                            xt[:, offs[ci]:offs[ci] + KT * nch]
                            .rearrange("p (k n) -> p k n", k=KT), mode))
                    xq.append(t)
                res[s] = {"w": w, "xq": xq, "offs": offs, "bb": s * BSLOT,
                          "h1": [[None] * NCH, [None] * NCH],
                          "e0ps": [None, None]}

            def e0_group(s, m, k):
                r = res[s]
                if k == 0:
                    r["e0ps"][m] = [ps_tile(128, nch) for nch in chunks]
                wk = r["w"][0:P, _E0 + k * 256 + 128 * m:
                            _E0 + k * 256 + 128 * m + 128]
                for ci, nch in enumerate(chunks):
                    nc.tensor.matmul(r["e0ps"][m][ci], wk,
                                     r["xq"][ci][0:P, k, :],
                                     start=(k == 0), stop=(k == KT - 1))
                if k == KT - 1:
                    for ci, nch in enumerate(chunks):
                        t = apool.tile([128, nch], mdt, tag="h1", name="h1",
                                       bufs=H1_BUFS)
                        drain_relu(t, r["e0ps"][m][ci], bias(128, r["bb"] + m))
                        r["h1"][m][ci] = t
                    r["e0ps"][m] = None

            E0_ORDER = [(m, k) for m in range(2) for k in range(KT)]

            if pipelined:
                ensure_slot(0)
                ensure_slot(1)
                r0 = res[0]
                for ci, nch in enumerate(chunks):
                    for m in range(2):
                        ps0 = ps_tile(128, nch)
                        for k in range(KT):
                            wk = r0["w"][0:P, _E0 + k * 256 + 128 * m:
                                         _E0 + k * 256 + 128 * m + 128]
                            nc.tensor.matmul(ps0, wk, r0["xq"][ci][0:P, k, :],
                                             start=(k == 0), stop=(k == KT - 1))
                        t = apool.tile([128, nch], mdt, tag="h1", name="h1",
                                       bufs=H1_BUFS)
                        drain_relu(t, ps0, bias(128, r0["bb"] + m))
                        r0["h1"][m][ci] = t

            for s in range(S):
                if pipelined:
                    ensure_slot(s + 2)
                    filler = iter(E0_ORDER) if s + 1 < S else iter([])
                else:
                    ensure_slot(s)
                    for m, k in E0_ORDER:
                        e0_group(s, m, k)
                    filler = iter([])

                def fill(n):
                    for _ in range(n):
                        mk = next(filler, None)
                        if mk is not None:
                            e0_group(s + 1, *mk)

                r = res[s]
                w, bb, offs, h1 = r["w"], r["bb"], r["offs"], r["h1"]

                ps = [None] * NCH
                for k in range(2):
                    wk = w[0:128, _E1 + 64 * k:_E1 + 64 * k + 64]
                    for ci, nch in enumerate(chunks):
                        if k == 0:
                            ps[ci] = ps_tile(64, nch)
                        nc.tensor.matmul(ps[ci], wk, h1[k][ci],
                                         start=(k == 0), stop=(k == 1))
                h2 = []
                for ci, nch in enumerate(chunks):
                    t = apool.tile([64, nch], mdt, tag="h2", name="h2", bufs=SM_BUFS)
                    drain_relu(t, ps[ci], bias(64, bb + 2))
                    h2.append(t)
                fill(2)

                ps = [None] * NCH
                wk = w[0:64, _E2:_E2 + 16]
                for ci, nch in enumerate(chunks):
                    ps[ci] = ps_tile(16, nch)
                    nc.tensor.matmul(ps[ci], wk, h2[ci], start=True, stop=True)
                z = []
                for ci, nch in enumerate(chunks):
                    t = apool.tile([16, nch], mdt, tag="z", name="z", bufs=SM_BUFS)
                    drain_relu(t, ps[ci], bias(16, bb + 3))
                    z.append(t)
                fill(2)

                ps = [None] * NCH
                wk = w[0:16, _D0:_D0 + 64]
                for ci, nch in enumerate(chunks):
                    ps[ci] = ps_tile(64, nch)
                    nc.tensor.matmul(ps[ci], wk, z[ci], start=True, stop=True)
                a1 = []
                for ci, nch in enumerate(chunks):
                    t = apool.tile([64, nch], mdt, tag="a1", name="a1", bufs=SM_BUFS)
                    drain_relu(t, ps[ci], bias(64, bb + 4))
                    a1.append(t)
                fill(2)

                a2 = [[None] * NCH, [None] * NCH]
                for m in range(2):
                    wk = w[0:64, _D1 + 128 * m:_D1 + 128 * m + 128]
                    ps = [None] * NCH
                    for ci, nch in enumerate(chunks):
                        ps[ci] = ps_tile(128, nch)
                        nc.tensor.matmul(ps[ci], wk, a1[ci],
                                         start=True, stop=True)
                    for ci, nch in enumerate(chunks):
                        t = apool.tile([128, nch], mdt, tag="a2", name="a2",
                                       bufs=7)
                        drain_relu(t, ps[ci], bias(128, bb + 5 + m))
                        a2[m][ci] = t
                    fill(2)

                yq = []
                for ci, nch in enumerate(chunks):
                    yq.append(iopool.tile([128, KT, nch], idt, tag="yq",
                                          name="yq", bufs=NCH + (3 if pipelined else 1)))
                for mm in range(KT):
                    ps = [None] * NCH
                    for k in range(2):
                        wk = w[0:128, _D2 + 784 * k + 112 * mm:
                               _D2 + 784 * k + 112 * mm + 112]
                        for ci, nch in enumerate(chunks):
                            if k == 0:
                                ps[ci] = ps_tile(112, nch)
                            nc.tensor.matmul(ps[ci], wk, a2[k][ci],
                                             start=(k == 0), stop=(k == 1))
                    for ci, nch in enumerate(chunks):
                        drain_bias(yq[ci][0:P, mm, :], ps[ci],
                                   bias(112, bb + 7 + mm))
                    if mm < 4:
                        fill(1)
                fill(14)
                for ci, nch in enumerate(chunks):
                    nc.sync.dma_start(
                        out=yt[:, offs[ci]:offs[ci] + KT * nch]
                        .rearrange("p (k n) -> p k n", k=KT),
                        in_=yq[ci][0:P])
                del res[s]
    nc.compile()
    return nc


_programs = {}


def _get_program(key, builder):
    if key not in _programs:
        _programs[key] = builder()
    return _programs[key]


def _pack_weights(params, slot_clusters):
    S = len(slot_clusters)
    wpk = np.zeros((128, S * WSLOT), np.float32)
    bpk = np.zeros((128, S * BSLOT), np.float32)
    for s, c in enumerate(slot_clusters):
        _pack_cluster(wpk, bpk, params, s, c)
    return wpk, bpk


def _route(labels, mode):
    counts = np.bincount(labels, minlength=K)
    configs = _CONFIGS if mode == "bf16" else _CONFIGS[1:]
    for S, R in configs:
        need = int(np.sum((counts + R - 1) // R))
        if need <= N_CORES * S:
            break
    nslots = N_CORES * S
    order = np.argsort(labels, kind="stable")
    slot_cluster = np.zeros(nslots, np.int64)
    slot_rows = [np.empty(0, np.int64)] * nslots
    si = pos = 0
    for c in range(K):
        cnt = int(counts[c])
        rows_c = order[pos:pos + cnt]
        pos += cnt
        for off in range(0, cnt, R):
            slot_cluster[si] = c
            slot_rows[si] = rows_c[off:off + R]
            si += 1
    return S, R, slot_cluster, slot_rows


def _flatten_xcore(xcore_t, R, chunks):
    ncols = xcore_t.shape[1]
    S = ncols // R
    flat = np.empty((P, ncols * KT), np.float32)
    pos = 0
    for s in range(S):
        col = s * R
        for nch in chunks:
            blk = xcore_t[:, col:col + nch]
            blk = blk.reshape(KT, P, nch).transpose(1, 0, 2)
            flat[:, pos:pos + KT * nch] = blk.reshape(P, KT * nch)
            pos += KT * nch
            col += nch
    return flat


def _unflatten_ycore(yflat, R, chunks):
    ncols = yflat.shape[1] // KT
    S = ncols // R
    out = np.empty((ncols, D), np.float32)
    pos = 0
    for s in range(S):
        col = s * R
        for nch in chunks:
            blk = yflat[:, pos:pos + KT * nch].reshape(P, KT, nch)
            out[col:col + nch] = blk.transpose(2, 1, 0).reshape(nch, D)
            pos += KT * nch
            col += nch
    return out


def _run_structured(x, params, strat, trace):
    import ml_dtypes
    caps, core_clusters, slot_rows = strat
    nc = _get_program(("st",) + tuple(caps),
                      lambda: _build_program_structured(caps))
    ncols = sum(caps)
    in_maps = []
    for i in range(N_CORES):
        xcore = np.zeros((ncols, D), np.float32)
        col = 0
        for s in range(len(caps)):
            rows = slot_rows[i][s]
            if len(rows):
                xcore[col:col + len(rows)] = x[rows]
            col += caps[s]
        wpk = np.zeros((128, 2 * WSLOT), np.float32)
        bpk = np.zeros((128, 2 * BSLOT), np.float32)
        _pack_cluster(wpk, bpk, params, 0, core_clusters[i][0])
        _pack_cluster(wpk, bpk, params, 1, core_clusters[i][1])
        xflat = _flatten_x_structured(np.ascontiguousarray(xcore.T), caps)
        in_maps.append({"xt": xflat.astype(ml_dtypes.bfloat16),
                        "wp": wpk.astype(ml_dtypes.bfloat16),
                        "bp": bpk})
    res = run_bass_kernel_spmd(nc, in_maps, core_ids=list(range(N_CORES)),
                               trace=trace)
    out = np.zeros_like(x)
    for i in range(N_CORES):
        yraw = np.asarray(res.results[i]["yt"]).astype(np.float32)
        ytT = _unflatten_y_structured(yraw, caps)
        col = 0
        for s in range(len(caps)):
            rows = slot_rows[i][s]
            if len(rows):
                out[rows] = ytT[col:col + len(rows)]
            col += caps[s]
    return out, res


def _run_generic(x, params, labels, mode, trace):
    S, R, slot_cluster, slot_rows = _route(labels, mode)
    chunks = _chunks(R, mode)
    nc = _get_program((S, R, mode), lambda: _build_program(S, R, mode))
    in_maps = []
    for i in range(N_CORES):
        xcore = np.zeros((S * R, D), np.float32)
        for s in range(S):
            rows = slot_rows[i * S + s]
            if len(rows):
                xcore[s * R: s * R + len(rows)] = x[rows]
        wpk, bpk = _pack_weights(params, slot_cluster[i * S:(i + 1) * S])
        xflat = _flatten_xcore(np.ascontiguousarray(xcore.T), R, chunks)
        if mode == "bf16":
            import ml_dtypes
            xflat = xflat.astype(ml_dtypes.bfloat16)
            wpk = wpk.astype(ml_dtypes.bfloat16)
        in_maps.append({"xt": xflat, "wp": wpk, "bp": bpk})
    res = run_bass_kernel_spmd(nc, in_maps, core_ids=list(range(N_CORES)),
                               trace=trace)
    out = np.zeros_like(x)
    for i in range(N_CORES):
        yraw = np.asarray(res.results[i]["yt"]).astype(np.float32)
        ytT = _unflatten_ycore(yraw, R, chunks)
        for s in range(S):
            rows = slot_rows[i * S + s]
            if len(rows):
                out[rows] = ytT[s * R: s * R + len(rows)]
    return out, res


def kernel_traced(inputs, trace=False, mode=None):
    if mode is None:
        mode = MODE
    x = np.ascontiguousarray(np.asarray(inputs["x"], dtype=np.float32))
    labels = np.asarray(inputs["kmeans_label"]).astype(np.int64).ravel()
    params = {k: np.asarray(v, dtype=np.float32)
              for k, v in inputs.items() if k not in ("x", "kmeans_label")}

    if mode == "bf16":
        strat = _route_structured(labels)
        if strat is not None:
            return _run_structured(x, params, strat, trace)
    return _run_generic(x, params, labels, mode, trace)


def kernel(**inputs):
    out, _ = kernel_traced(inputs, trace=False)
    return out


# revision 14
# speedup vs baseline: 1.0793x; 1.0286x over previous
"""MoE-routed K-cluster autoencoder kernel for 8 Trainium2 NeuronCores.

Strategy
--------
Each row of x is reconstructed by the autoencoder of its kmeans cluster.
Computing all K experts densely for every row (like the reference) does
10x the needed matmul work, so we *route*.

Structured path (default for ~uniform labels): the label histogram is
known at call time, so slot capacities are fitted to it.  The two
largest clusters are split 4 ways into the per-core "col4" slot (4 cores
each); the remaining 8 clusters are each *owned* by one core and span
that core's slots 0-2, which share a single weight load.  Per-core slot
capacities (1024, 1024, M-2048, ceil(maxbig/4)) give ~4170 row-slots vs
4608 for the old fixed-capacity config (-10% PE work and x/y bytes), and
owning one cluster per core halves the weight DMA (2 unique clusters per
core instead of 4).  Chunks are ~512 columns (one PSUM bank) to minimize
LDWEIGHTS re-issues and PSUM->SBUF eviction instruction count.  The
decoder output is drained stripe-major and each 112-feature stripe is
DMA'd out as soon as it is evicted, so the final slot exposes only one
stripe's DMA instead of the whole slot.  Slot 0's first x chunk lands in
two k-split DMAs so e0 can start after ~half the chunk arrives.

Fallback path (skewed/degenerate labels): the original fixed-capacity
slot config search, unchanged.

Device pipeline (both paths): per slot, the 6-layer MLP chain runs as
feature-major matmuls (outT = W.T @ actsT); the next slot's encoder-0
work is software-pipelined into the current slot's serial mid-layers;
PSUM->SBUF bias+ReLU evictions alternate between ScalarE and VectorE; a
short pre-warm matmul burst opens the HAM clock gate while the first
DMAs land.  bf16 operands end-to-end (~5.6e-3 scale-relative error).
"""

import numpy as np

import concourse.tile as tile
from concourse import bacc, mybir
from concourse.bass_utils import run_bass_kernel_spmd

N_CORES = 8
B, D, H1, H2, L, K = 32768, 784, 256, 64, 16, 10
P = 112          # partition tile height for the D axis: 784 = 7 * 112
KT = D // P      # 7 k-tiles along D

# packed weight layout (column offsets in a [128, WSLOT] block)
_E0, _E1, _E2, _D0, _D1, _D2 = 0, 1792, 1920, 1936, 2000, 2256
WSLOT = 3824     # = 7*256 + 2*64 + 16 + 64 + 256 + 2*784
BSLOT = 14       # bias columns per block: 2 + 1 + 1 + 1 + 2 + 7

# (slots_per_core, rows_per_slot) fallback configs
_CONFIGS = [(4, 1152), (4, 1280), (8, 640), (16, 320), (32, 160)]

_F32 = mybir.dt.float32
_F32R = mybir.dt.float32r
_BF16 = mybir.dt.bfloat16
_RELU = mybir.ActivationFunctionType.Relu

MODE = "bf16"


def _slot_chunks(C):
    """Split C columns into equal-ish chunks of <=512 (one PSUM bank)."""
    n = max(1, (C + 511) // 512)
    base, extra = divmod(C, n)
    return [base + (1 if i < extra else 0) for i in range(n)]


# ---------------------------------------------------------------------------
# structured program: caps per slot, slots 0-2 share weight block 0,
# slot 3 uses weight block 1.  x chunk-flattened; y stripe-major.
# ---------------------------------------------------------------------------

def _build_program_structured(caps):
    S = len(caps)
    chunk_lists = [_slot_chunks(c) for c in caps]
    XQ_BUFS = sum(len(cl) for cl in chunk_lists)
    nflat = KT * sum(caps)
    nc = bacc.Bacc("TRN2", target_bir_lowering=False, debug=False)
    xt = nc.dram_tensor("xt", [P, nflat], _BF16, kind="ExternalInput").ap()
    wp = nc.dram_tensor("wp", [128, 2 * WSLOT], _BF16, kind="ExternalInput").ap()
    bp = nc.dram_tensor("bp", [128, 2 * BSLOT], _F32, kind="ExternalInput").ap()
    yt = nc.dram_tensor("yt", [P, nflat], _BF16, kind="ExternalOutput").ap()

    # per-slot x column offsets (in xt/yt, units of columns)
    slot_off = []
    cum = 0
    for c in caps:
        slot_off.append(cum)
        cum += KT * c

    # slot -> weight/bias block: slots 0-2 share the owned cluster's
    # weights; the remaining slot(s) hold the big cluster's
    wblk = [0, 0, 0] + [1] * (S - 3)

    with tile.TileContext(nc) as tc:
        with (
            tc.tile_pool(name="wpool", bufs=1) as wpool,
            tc.tile_pool(name="iopool", bufs=1) as iopool,
            tc.tile_pool(name="apool", bufs=1) as apool,
            tc.tile_pool(name="pspool", bufs=1, space="PSUM") as pspool,
        ):
            bsb = wpool.tile([128, 2 * BSLOT], _F32, tag="b", name="bsb", bufs=1)
            wA = wpool.tile([128, WSLOT], _BF16, tag="wA", name="wA", bufs=1)
            wB = wpool.tile([128, WSLOT], _BF16, tag="wB", name="wB", bufs=1)

            xq = {}  # (s, ci) -> tile

            def fetch_slot_x(s, eng, ksplit=False):
                if s >= S:
                    return
                off = slot_off[s]
                for ci, nch in enumerate(chunk_lists[s]):
                    t = iopool.tile([128, KT, nch], _BF16, tag="xq",
                                    name="xq", bufs=XQ_BUFS)
                    src = xt[:, off:off + KT * nch].rearrange(
                        "p (k n) -> p k n", k=KT)
                    if ksplit and ci == 0:
                        # subtile deps let e0's k<4 matmuls start after
                        # the first half of the chunk lands
                        eng.dma_start(out=t[0:P, 0:4], in_=src[:, 0:4])
                        eng.dma_start(out=t[0:P, 4:KT], in_=src[:, 4:KT])
                    else:
                        eng.dma_start(out=t[0:P], in_=src)
                    xq[(s, ci)] = t
                    off += KT * nch

            # The kernel runs close to the per-core DMA throughput
            # ceiling, so ALL x tiles are fetched up front (they fit in
            # SBUF) -- the queues stay saturated from t=0 and the e0
            # units never wait on a just-in-time prefetch.  Enqueues
            # (~0.6us each, serialized per engine) are split across the
            # two HWDGE engines so they don't gate the first e0.
            fetch_slot_x(0, nc.sync, ksplit=True)
            nc.scalar.dma_start(out=wA[:, 0:_E1], in_=wp[:, 0:_E1])
            nc.sync.dma_start(out=bsb, in_=bp)
            nc.scalar.dma_start(out=wA[:, _E1:], in_=wp[:, _E1:WSLOT])
            fetch_slot_x(1, nc.sync)
            nc.scalar.dma_start(out=wB, in_=wp[:, WSLOT:2 * WSLOT])
            fetch_slot_x(2, nc.sync)
            fetch_slot_x(3, nc.scalar)
            for s5 in range(4, S):
                fetch_slot_x(s5, nc.sync)

            # PE pre-warm: a few throwaway matmuls get PE activity started
            # (HAM clock-gate) while the first DMAs land, without delaying
            # e0 behind a long in-order warmup burst.
            wu = wpool.tile([128, 512], _BF16, tag="wu", name="wu", bufs=1)
            nc.gpsimd.memset(wu, 0)
            wups = [pspool.tile([128, 512], _F32, tag="ps", name="wups",
                                bufs=8) for _ in range(4)]
            for i in range(6):
                nc.tensor.matmul(wups[i % 4], wu[:, 0:128], wu,
                                 start=True, stop=True)

            def wt(s):
                return wA if wblk[s] == 0 else wB

            def bias(s, lo, col):
                bb = wblk[s] * BSLOT
                return bsb[0:lo, bb + col:bb + col + 1]

            def ps_tile(parts, nch):
                return pspool.tile([parts, nch], _F32, tag="ps", name="ps",
                                   bufs=8)

            drain_i = [0]

            def drain_relu(out, ps, bias_ap):
                drain_i[0] += 1
                if drain_i[0] % 2:
                    nc.scalar.activation(out, ps, _RELU, bias=bias_ap)
                else:
                    nc.vector.tensor_scalar(out, ps, bias_ap, 0.0,
                                            mybir.AluOpType.add,
                                            mybir.AluOpType.max)

            def drain_bias(out, ps, bias_ap):
                drain_i[0] += 1
                if drain_i[0] % 2:
                    nc.scalar.add(out, ps, bias_ap)
                else:
                    nc.vector.tensor_scalar_add(out, ps, bias_ap)

            h1s = {}   # s -> [m][ci] h1 tiles
            a2s = {}   # s -> [m][ci] a2 tiles

            def e0_unit(s, ci, m):
                """One e0 unit: chunk ci, m-half m, all 7 k accumulations."""
                nch = chunk_lists[s][ci]
                w = wt(s)
                ps = ps_tile(128, nch)
                for k in range(KT):
                    wk = w[0:P, _E0 + k * 256 + 128 * m:
                           _E0 + k * 256 + 128 * m + 128]
                    nc.tensor.matmul(ps, wk, xq[(s, ci)][0:P, k, :],
                                     start=(k == 0), stop=(k == KT - 1))
                t = apool.tile([128, nch], _BF16, tag="h1", name="h1",
                               bufs=14)
                drain_relu(t, ps, bias(s, 128, m))
                h1s.setdefault(s, [{}, {}])[m][ci] = t

            def mid_steps(s):
                """The serial e1->e2->d0->d1 chain of slot s as 5 steps.

                Consecutive steps must be separated by ~1us of unrelated
                PE work (a d2 stripe of the previous slot) to hide the
                PSUM-eviction latency between layers."""
                if s >= S:
                    return
                chunks = chunk_lists[s]
                NCH = len(chunks)
                w = wt(s)
                st = {}

                def e1():
                    h1 = h1s[s]
                    ps = [None] * NCH
                    for k in range(2):
                        wk = w[0:128, _E1 + 64 * k:_E1 + 64 * k + 64]
                        for ci, nch in enumerate(chunks):
                            if k == 0:
                                ps[ci] = ps_tile(64, nch)
                            nc.tensor.matmul(ps[ci], wk, h1[k][ci],
                                             start=(k == 0), stop=(k == 1))
                    st["h2"] = []
                    for ci, nch in enumerate(chunks):
                        t = apool.tile([64, nch], _BF16, tag="h2", name="h2",
                                       bufs=4)
                        drain_relu(t, ps[ci], bias(s, 64, 2))
                        st["h2"].append(t)

                def e2():
                    ps = [None] * NCH
                    wk = w[0:64, _E2:_E2 + 16]
                    for ci, nch in enumerate(chunks):
                        ps[ci] = ps_tile(16, nch)
                        nc.tensor.matmul(ps[ci], wk, st["h2"][ci],
                                         start=True, stop=True)
                    st["z"] = []
                    for ci, nch in enumerate(chunks):
                        t = apool.tile([16, nch], _BF16, tag="z", name="z",
                                       bufs=4)
                        drain_relu(t, ps[ci], bias(s, 16, 3))
                        st["z"].append(t)

                def d0():
                    ps = [None] * NCH
                    wk = w[0:16, _D0:_D0 + 64]
                    for ci, nch in enumerate(chunks):
                        ps[ci] = ps_tile(64, nch)
                        nc.tensor.matmul(ps[ci], wk, st["z"][ci],
                                         start=True, stop=True)
                    st["a1"] = []
                    for ci, nch in enumerate(chunks):
                        t = apool.tile([64, nch], _BF16, tag="a1", name="a1",
                                       bufs=4)
                        drain_relu(t, ps[ci], bias(s, 64, 4))
                        st["a1"].append(t)

                def d1(m):
                    a2 = a2s.setdefault(s, [[None] * NCH, [None] * NCH])
                    wk = w[0:64, _D1 + 128 * m:_D1 + 128 * m + 128]
                    ps = [None] * NCH
                    for ci, nch in enumerate(chunks):
                        ps[ci] = ps_tile(128, nch)
                        nc.tensor.matmul(ps[ci], wk, st["a1"][ci],
                                         start=True, stop=True)
                    for ci, nch in enumerate(chunks):
                        t = apool.tile([128, nch], _BF16, tag="a2", name="a2",
                                       bufs=13)
                        drain_relu(t, ps[ci], bias(s, 128, 5 + m))
                        a2[m][ci] = t

                yield e1
                yield e2
                yield d0
                yield lambda: d1(0)
                yield lambda: d1(1)

            def e0_unit_steps(s):
                if s >= S:
                    return
                for ci in range(len(chunk_lists[s])):
                    for m in range(2):
                        yield (lambda ci=ci, m=m: e0_unit(s, ci, m))

            # ---- prologue: e0(0) inline with warmup matmuls between the
            # units (keeps PE activity sustained through the head DMA ramp
            # so the HAM clock gate opens as early as possible); mid(0)
            # interleaved with e0(1)
            wui = [0]

            def warm(n):
                for _ in range(n):
                    nc.tensor.matmul(wups[wui[0] % 4], wu[:, 0:128], wu,
                                     start=True, stop=True)
                    wui[0] += 1

            for ci in range(len(chunk_lists[0])):
                for m in range(2):
                    e0_unit(0, ci, m)
                    if ci == 0:
                        warm(2)
            u1 = e0_unit_steps(1)
            for step in mid_steps(0):
                step()
                u = next(u1, None)
                if u is not None:
                    u()
            for u in u1:
                u()

            # ---- main pipeline: iteration i runs d2(i) stripes with
            # mid(i+1) steps and e0(i+2) units interleaved between stripes
            for i in range(S):
                chunks = chunk_lists[i]
                NCH = len(chunks)
                w = wt(i)
                steps = list(mid_steps(i + 1))
                units = list(e0_unit_steps(i + 2))
                a2 = a2s[i]
                col_off = []
                c0 = 0
                for nch in chunks:
                    col_off.append(c0)
                    c0 += nch
                Cs = caps[i]
                yqs = None
                for mm in range(KT):
                    half = mm % 2
                    if half == 0:
                        nst = 1 if mm == KT - 1 else 2
                        yqs = iopool.tile([112, nst, Cs], _BF16, tag="yq",
                                          name="yqs", bufs=6)
                    ps = [None] * NCH
                    for k in range(2):
                        wk = w[0:128, _D2 + 784 * k + 112 * mm:
                               _D2 + 784 * k + 112 * mm + 112]
                        for ci, nch in enumerate(chunks):
                            if k == 0:
                                ps[ci] = ps_tile(112, nch)
                            nc.tensor.matmul(ps[ci], wk, a2[k][ci],
                                             start=(k == 0), stop=(k == 1))
                    for ci, nch in enumerate(chunks):
                        drain_bias(yqs[0:P, half,
                                       col_off[ci]:col_off[ci] + nch],
                                   ps[ci], bias(i, 112, 7 + mm))
                    if half == 1 or mm == KT - 1:
                        lo = (mm // 2) * 2
                        nc.sync.dma_start(
                            out=yt[:, slot_off[i] + lo * Cs:
                                   slot_off[i] + (mm + 1) * Cs]
                            .rearrange("p (t n) -> p t n", n=Cs),
                            in_=yqs[0:P])
                    if mm < KT - 1:
                        # one mid step of slot i+1 per stripe boundary
                        # (stripe work hides the eviction latency), then
                        # e0(i+2) units once the chain is emitted
                        if steps:
                            steps.pop(0)()
                        elif units:
                            units.pop(0)()
                            if units:
                                units.pop(0)()
                for step in steps:
                    step()
                for u in units:
                    u()
    nc.compile()
    return nc


def _route_structured(labels):
    """Fit (1024, 1024, M-2048, ceil(maxbig/4)) caps to the histogram.

    Returns (caps, core_clusters[8][2], slot_rows[8][4]) or None if the
    distribution doesn't fit the structured layout."""
    counts = np.bincount(labels, minlength=K)
    if len(counts) != K:
        return None
    order = np.argsort(counts, kind="stable")[::-1]
    b1, b2 = int(order[0]), int(order[1])
    owned = [int(c) for c in order[2:]]
    if len(owned) != 8:
        return None
    M = int(counts[owned].max())
    Mb = int(counts[b1])
    C3 = M - 2048
    C4 = -(-Mb // 4)
    if C3 < 256 or C3 > 2048 or C4 < 256 or C4 > 1024:
        return None
    caps = (1024, 1024, C3, C4)
    if sum(caps) >= 4608:
        return None

    by_cluster = {}
    srt = np.argsort(labels, kind="stable")
    pos = 0
    for c in np.sort(np.unique(labels)):
        cnt = int(counts[c])
        by_cluster[int(c)] = srt[pos:pos + cnt]
        pos += cnt

    core_clusters = []
    slot_rows = []
    for i in range(N_CORES):
        oc = owned[i]
        bc = b1 if i < 4 else b2
        core_clusters.append((oc, bc))
        rows = by_cluster.get(oc, np.empty(0, np.int64))
        srows = [rows[0:1024], rows[1024:2048], rows[2048:]]
        brows = by_cluster.get(bc, np.empty(0, np.int64))
        j = i % 4
        q, r = divmod(len(brows), 4)
        starts = [qq * q + min(qq, r) for qq in range(5)]
        part = brows[starts[j]:starts[j + 1]]
        if len(caps) == 5:
            srows.append(part[:caps[3]])
            srows.append(part[caps[3]:])
        else:
            srows.append(part)
        for s in range(len(caps)):
            if len(srows[s]) > caps[s]:
                return None
        slot_rows.append(srows)
    return caps, core_clusters, slot_rows


def _pack_cluster(wpk, bpk, params, blk, c):
    """Pack cluster c's weights/biases into block blk of wpk/bpk."""
    wb, bb = blk * WSLOT, blk * BSLOT
    we0, we1, we2 = params["w_e0"][c], params["w_e1"][c], params["w_e2"][c]
    wd0, wd1, wd2 = params["w_d0"][c], params["w_d1"][c], params["w_d2"][c]
    for k in range(KT):
        wpk[0:P, wb + _E0 + k * 256: wb + _E0 + (k + 1) * 256] = \
            we0[P * k:P * (k + 1), :]
    for k in range(2):
        wpk[0:128, wb + _E1 + 64 * k: wb + _E1 + 64 * (k + 1)] = \
            we1[128 * k:128 * (k + 1), :]
    wpk[0:64, wb + _E2: wb + _E2 + 16] = we2
    wpk[0:16, wb + _D0: wb + _D0 + 64] = wd0
    wpk[0:64, wb + _D1: wb + _D1 + 256] = wd1
    for k in range(2):
        wpk[0:128, wb + _D2 + 784 * k: wb + _D2 + 784 * (k + 1)] = \
            wd2[128 * k:128 * (k + 1), :]
    be0, be1, be2 = params["b_e0"][c], params["b_e1"][c], params["b_e2"][c]
    bd0, bd1, bd2 = params["b_d0"][c], params["b_d1"][c], params["b_d2"][c]
    bpk[0:128, bb + 0] = be0[0:128]
    bpk[0:128, bb + 1] = be0[128:256]
    bpk[0:64, bb + 2] = be1
    bpk[0:16, bb + 3] = be2
    bpk[0:64, bb + 4] = bd0
    bpk[0:128, bb + 5] = bd1[0:128]
    bpk[0:128, bb + 6] = bd1[128:256]
    for m in range(KT):
        bpk[0:P, bb + 7 + m] = bd2[P * m:P * (m + 1)]


def _flatten_x_structured(xcore_t, caps):
    """[D, sum(caps)] feature-major slab -> chunk-flattened [P, KT*sum]."""
    flat = np.empty((P, KT * sum(caps)), np.float32)
    pos = col = 0
    for C in caps:
        for nch in _slot_chunks(C):
            blk = xcore_t[:, col:col + nch]                   # [784, nch]
            blk = blk.reshape(KT, P, nch).transpose(1, 0, 2)  # [P, KT, nch]
            flat[:, pos:pos + KT * nch] = blk.reshape(P, KT * nch)
            pos += KT * nch
            col += nch
    return flat


def _unflatten_y_structured(yflat, caps):
    """stripe-major [P, KT*sum(caps)] -> row-major [sum(caps), D]."""
    out = np.empty((sum(caps), D), np.float32)
    pos = col = 0
    for C in caps:
        blk = yflat[:, pos:pos + KT * C].reshape(P, KT, C)
        out[col:col + C] = blk.transpose(2, 1, 0).reshape(C, D)
        pos += KT * C
        col += C
    return out


# ---------------------------------------------------------------------------
# fallback path: original fixed-capacity config program (unchanged)
# ---------------------------------------------------------------------------

def _mdt_view(ap, mode):
    return ap.bitcast(_F32R) if mode == "f32r" else ap


def _chunks(R, mode="f32r"):
    if mode == "bf16":
        n = max(1, (R + 511) // 512)
        base, extra = divmod(R, n)
        return [base + (1 if i < extra else 0) for i in range(n)]
    out, rem = [], R
    while rem > 0:
        c = min(512, rem)
        if c == 512 and 0 < rem - c < 256:
            c = max(256, min(512, (rem + 1) // 2))
        out.append(c)
        rem -= c
    return out


def _build_program(S, R, mode):
    mdt = _F32R if mode == "f32r" else _BF16
    idt = _F32 if mode == "f32r" else _BF16
    pipelined = mode == "bf16"
    ncols = S * R
    nflat = ncols * KT
    nc = bacc.Bacc("TRN2", target_bir_lowering=False, debug=False)
    xt = nc.dram_tensor("xt", [P, nflat], idt, kind="ExternalInput").ap()
    wp = nc.dram_tensor("wp", [128, S * WSLOT], idt, kind="ExternalInput").ap()
    bp = nc.dram_tensor("bp", [128, S * BSLOT], _F32, kind="ExternalInput").ap()
    yt = nc.dram_tensor("yt", [P, nflat], idt, kind="ExternalOutput").ap()

    chunks = _chunks(R, mode)
    NCH = len(chunks)
    XQ_BUFS = 3 * NCH if pipelined else NCH + 2
    W_BUFS = 4 if pipelined else 2
    H1_BUFS = 10 if pipelined else 6
    SM_BUFS = 6 if pipelined else 3

    with tile.TileContext(nc) as tc:
        with (
            tc.tile_pool(name="wpool", bufs=1) as wpool,
            tc.tile_pool(name="iopool", bufs=1) as iopool,
            tc.tile_pool(name="apool", bufs=1) as apool,
            tc.tile_pool(name="pspool", bufs=1, space="PSUM") as pspool,
        ):
            bsb = wpool.tile([128, S * BSLOT], _F32, tag="b", name="bsb", bufs=1)
            nc.sync.dma_start(out=bsb, in_=bp)

            wu = wpool.tile([128, 512], _BF16, tag="wu", name="wu", bufs=1)
            nc.vector.memset(wu, 0)
            wups = [pspool.tile([128, 512], _F32, tag="ps", name="wups",
                                bufs=8) for _ in range(4)]
            for i in range(16):
                nc.tensor.matmul(wups[i % 4], wu[:, 0:128], wu,
                                 start=True, stop=True)

            def bias(lo, col):
                return bsb[0:lo, col:col + 1]

            def ps_tile(parts, nch):
                return pspool.tile([parts, nch], _F32, tag="ps", name="ps",
                                   bufs=8)

            drain_i = [0]

            def drain_relu(out, ps, bias_ap):
                drain_i[0] += 1
                if drain_i[0] % 2:
                    nc.scalar.activation(out, ps, _RELU, bias=bias_ap)
                else:
                    nc.vector.tensor_scalar(out, ps, bias_ap, 0.0,
                                            mybir.AluOpType.add,
                                            mybir.AluOpType.max)

            def drain_bias(out, ps, bias_ap):
                drain_i[0] += 1
                if drain_i[0] % 2:
                    nc.scalar.add(out, ps, bias_ap)
                else:
                    nc.vector.tensor_scalar_add(out, ps, bias_ap)

            res = {}

            def ensure_slot(s):
                if s in res or s >= S:
                    return
                w = wpool.tile([128, WSLOT], mdt, tag="w", name="w",
                               bufs=W_BUFS)
                nc.sync.dma_start(
                    out=w[:, 0:_E1],
                    in_=_mdt_view(wp[:, s * WSLOT:s * WSLOT + _E1], mode))
                nc.sync.dma_start(
                    out=w[:, _E1:],
                    in_=_mdt_view(wp[:, s * WSLOT + _E1:(s + 1) * WSLOT], mode))
                offs = []
                cum = s * R * KT
                for nch in chunks:
                    offs.append(cum)
                    cum += nch * KT
                xq = []
                for ci, nch in enumerate(chunks):
                    t = iopool.tile([128, KT, nch], mdt, tag="xq", name="xq",
                                    bufs=XQ_BUFS)
                    nc.sync.dma_start(
                        out=t[0:P],
                        in_=_mdt_view(
                            xt[:, offs[ci]:offs[ci] + KT * nch]
                            .rearrange("p (k n) -> p k n", k=KT), mode))
                    xq.append(t)
                res[s] = {"w": w, "xq": xq, "offs": offs, "bb": s * BSLOT,
                          "h1": [[None] * NCH, [None] * NCH],
                          "e0ps": [None, None]}

            def e0_group(s, m, k):
                r = res[s]
                if k == 0:
                    r["e0ps"][m] = [ps_tile(128, nch) for nch in chunks]
                wk = r["w"][0:P, _E0 + k * 256 + 128 * m:
                            _E0 + k * 256 + 128 * m + 128]
                for ci, nch in enumerate(chunks):
                    nc.tensor.matmul(r["e0ps"][m][ci], wk,
                                     r["xq"][ci][0:P, k, :],
                                     start=(k == 0), stop=(k == KT - 1))
                if k == KT - 1:
                    for ci, nch in enumerate(chunks):
                        t = apool.tile([128, nch], mdt, tag="h1", name="h1",
                                       bufs=H1_BUFS)
                        drain_relu(t, r["e0ps"][m][ci], bias(128, r["bb"] + m))
                        r["h1"][m][ci] = t
                    r["e0ps"][m] = None

            E0_ORDER = [(m, k) for m in range(2) for k in range(KT)]

            if pipelined:
                ensure_slot(0)
                ensure_slot(1)
                r0 = res[0]
                for ci, nch in enumerate(chunks):
                    for m in range(2):
                        ps0 = ps_tile(128, nch)
                        for k in range(KT):
                            wk = r0["w"][0:P, _E0 + k * 256 + 128 * m:
                                         _E0 + k * 256 + 128 * m + 128]
                            nc.tensor.matmul(ps0, wk, r0["xq"][ci][0:P, k, :],
                                             start=(k == 0), stop=(k == KT - 1))
                        t = apool.tile([128, nch], mdt, tag="h1", name="h1",
                                       bufs=H1_BUFS)
                        drain_relu(t, ps0, bias(128, r0["bb"] + m))
                        r0["h1"][m][ci] = t

            for s in range(S):
                if pipelined:
                    ensure_slot(s + 2)
                    filler = iter(E0_ORDER) if s + 1 < S else iter([])
                else:
                    ensure_slot(s)
                    for m, k in E0_ORDER:
                        e0_group(s, m, k)
                    filler = iter([])

                def fill(n):
                    for _ in range(n):
                        mk = next(filler, None)
                        if mk is not None:
                            e0_group(s + 1, *mk)

                r = res[s]
                w, bb, offs, h1 = r["w"], r["bb"], r["offs"], r["h1"]

                ps = [None] * NCH
                for k in range(2):
                    wk = w[0:128, _E1 + 64 * k:_E1 + 64 * k + 64]
                    for ci, nch in enumerate(chunks):
                        if k == 0:
                            ps[ci] = ps_tile(64, nch)
                        nc.tensor.matmul(ps[ci], wk, h1[k][ci],
                                         start=(k == 0), stop=(k == 1))
                h2 = []
                for ci, nch in enumerate(chunks):
                    t = apool.tile([64, nch], mdt, tag="h2", name="h2", bufs=SM_BUFS)
                    drain_relu(t, ps[ci], bias(64, bb + 2))
                    h2.append(t)
                fill(2)

                ps = [None] * NCH
                wk = w[0:64, _E2:_E2 + 16]
                for ci, nch in enumerate(chunks):
                    ps[ci] = ps_tile(16, nch)
                    nc.tensor.matmul(ps[ci], wk, h2[ci], start=True, stop=True)
                z = []
                for ci, nch in enumerate(chunks):
                    t = apool.tile([16, nch], mdt, tag="z", name="z", bufs=SM_BUFS)
                    drain_relu(t, ps[ci], bias(16, bb + 3))
                    z.append(t)
                fill(2)

                ps = [None] * NCH
                wk = w[0:16, _D0:_D0 + 64]
                for ci, nch in enumerate(chunks):
                    ps[ci] = ps_tile(64, nch)
                    nc.tensor.matmul(ps[ci], wk, z[ci], start=True, stop=True)
                a1 = []
                for ci, nch in enumerate(chunks):
                    t = apool.tile([64, nch], mdt, tag="a1", name="a1", bufs=SM_BUFS)
                    drain_relu(t, ps[ci], bias(64, bb + 4))
                    a1.append(t)
                fill(2)

                a2 = [[None] * NCH, [None] * NCH]
                for m in range(2):
                    wk = w[0:64, _D1 + 128 * m:_D1 + 128 * m + 128]
                    ps = [None] * NCH
                    for ci, nch in enumerate(chunks):
                        ps[ci] = ps_tile(128, nch)
                        nc.tensor.matmul(ps[ci], wk, a1[ci],
                                         start=True, stop=True)
                    for ci, nch in enumerate(chunks):
                        t = apool.tile([128, nch], mdt, tag="a2", name="a2",
                                       bufs=7)
                        drain_relu(t, ps[ci], bias(128, bb + 5 + m))
                        a2[m][ci] = t
                    fill(2)

                yq = []
                for ci, nch in enumerate(chunks):
                    yq.append(iopool.tile([128, KT, nch], idt, tag="yq",
                                          name="yq", bufs=NCH + (3 if pipelined else 1)))
                for mm in range(KT):
                    ps = [None] * NCH
                    for k in range(2):
                        wk = w[0:128, _D2 + 784 * k + 112 * mm:
                               _D2 + 784 * k + 112 * mm + 112]
                        for ci, nch in enumerate(chunks):
                            if k == 0:
                                ps[ci] = ps_tile(112, nch)
                            nc.tensor.matmul(ps[ci], wk, a2[k][ci],
                                             start=(k == 0), stop=(k == 1))
                    for ci, nch in enumerate(chunks):
                        drain_bias(yq[ci][0:P, mm, :], ps[ci],
                                   bias(112, bb + 7 + mm))
                    if mm < 4:
                        fill(1)
                fill(14)
                for ci, nch in enumerate(chunks):
                    nc.sync.dma_start(
                        out=yt[:, offs[ci]:offs[ci] + KT * nch]
                        .rearrange("p (k n) -> p k n", k=KT),
                        in_=yq[ci][0:P])
                del res[s]
    nc.compile()
    return nc


_programs = {}


def _get_program(key, builder):
    if key not in _programs:
        _programs[key] = builder()
    return _programs[key]


def _pack_weights(params, slot_clusters):
    S = len(slot_clusters)
    wpk = np.zeros((128, S * WSLOT), np.float32)
    bpk = np.zeros((128, S * BSLOT), np.float32)
    for s, c in enumerate(slot_clusters):
        _pack_cluster(wpk, bpk, params, s, c)
    return wpk, bpk


def _route(labels, mode):
    counts = np.bincount(labels, minlength=K)
    configs = _CONFIGS if mode == "bf16" else _CONFIGS[1:]
    for S, R in configs:
        need = int(np.sum((counts + R - 1) // R))
        if need <= N_CORES * S:
            break
    nslots = N_CORES * S
    order = np.argsort(labels, kind="stable")
    slot_cluster = np.zeros(nslots, np.int64)
    slot_rows = [np.empty(0, np.int64)] * nslots
    si = pos = 0
    for c in range(K):
        cnt = int(counts[c])
        rows_c = order[pos:pos + cnt]
        pos += cnt
        for off in range(0, cnt, R):
            slot_cluster[si] = c
            slot_rows[si] = rows_c[off:off + R]
            si += 1
    return S, R, slot_cluster, slot_rows


def _flatten_xcore(xcore_t, R, chunks):
    ncols = xcore_t.shape[1]
    S = ncols // R
    flat = np.empty((P, ncols * KT), np.float32)
    pos = 0
    for s in range(S):
        col = s * R
        for nch in chunks:
            blk = xcore_t[:, col:col + nch]
            blk = blk.reshape(KT, P, nch).transpose(1, 0, 2)
            flat[:, pos:pos + KT * nch] = blk.reshape(P, KT * nch)
            pos += KT * nch
            col += nch
    return flat


def _unflatten_ycore(yflat, R, chunks):
    ncols = yflat.shape[1] // KT
    S = ncols // R
    out = np.empty((ncols, D), np.float32)
    pos = 0
    for s in range(S):
        col = s * R
        for nch in chunks:
            blk = yflat[:, pos:pos + KT * nch].reshape(P, KT, nch)
            out[col:col + nch] = blk.transpose(2, 1, 0).reshape(nch, D)
            pos += KT * nch
            col += nch
    return out


def _run_structured(x, params, strat, trace):
    import ml_dtypes
    caps, core_clusters, slot_rows = strat
    nc = _get_program(("st",) + tuple(caps),
                      lambda: _build_program_structured(caps))
    ncols = sum(caps)
    in_maps = []
    for i in range(N_CORES):
        xcore = np.zeros((ncols, D), np.float32)
        col = 0
        for s in range(len(caps)):
            rows = slot_rows[i][s]
            if len(rows):
                xcore[col:col + len(rows)] = x[rows]
            col += caps[s]
        wpk = np.zeros((128, 2 * WSLOT), np.float32)
        bpk = np.zeros((128, 2 * BSLOT), np.float32)
        _pack_cluster(wpk, bpk, params, 0, core_clusters[i][0])
        _pack_cluster(wpk, bpk, params, 1, core_clusters[i][1])
        xflat = _flatten_x_structured(np.ascontiguousarray(xcore.T), caps)
        in_maps.append({"xt": xflat.astype(ml_dtypes.bfloat16),
                        "wp": wpk.astype(ml_dtypes.bfloat16),
                        "bp": bpk})
    res = run_bass_kernel_spmd(nc, in_maps, core_ids=list(range(N_CORES)),
                               trace=trace)
    out = np.zeros_like(x)
    for i in range(N_CORES):
        yraw = np.asarray(res.results[i]["yt"]).astype(np.float32)
        ytT = _unflatten_y_structured(yraw, caps)
        col = 0
        for s in range(len(caps)):
            rows = slot_rows[i][s]
            if len(rows):
                out[rows] = ytT[col:col + len(rows)]
            col += caps[s]
    return out, res


def _run_generic(x, params, labels, mode, trace):
    S, R, slot_cluster, slot_rows = _route(labels, mode)
    chunks = _chunks(R, mode)
    nc = _get_program((S, R, mode), lambda: _build_program(S, R, mode))
    in_maps = []
    for i in range(N_CORES):
        xcore = np.zeros((S * R, D), np.float32)
        for s in range(S):
            rows = slot_rows[i * S + s]
            if len(rows):
                xcore[s * R: s * R + len(rows)] = x[rows]
        wpk, bpk = _pack_weights(params, slot_cluster[i * S:(i + 1) * S])
        xflat = _flatten_xcore(np.ascontiguousarray(xcore.T), R, chunks)
        if mode == "bf16":
            import ml_dtypes
            xflat = xflat.astype(ml_dtypes.bfloat16)
            wpk = wpk.astype(ml_dtypes.bfloat16)
        in_maps.append({"xt": xflat, "wp": wpk, "bp": bpk})
    res = run_bass_kernel_spmd(nc, in_maps, core_ids=list(range(N_CORES)),
                               trace=trace)
    out = np.zeros_like(x)
    for i in range(N_CORES):
        yraw = np.asarray(res.results[i]["yt"]).astype(np.float32)
        ytT = _unflatten_ycore(yraw, R, chunks)
        for s in range(S):
            rows = slot_rows[i * S + s]
            if len(rows):
                out[rows] = ytT[s * R: s * R + len(rows)]
    return out, res


def kernel_traced(inputs, trace=False, mode=None):
    if mode is None:
        mode = MODE
    x = np.ascontiguousarray(np.asarray(inputs["x"], dtype=np.float32))
    labels = np.asarray(inputs["kmeans_label"]).astype(np.int64).ravel()
    params = {k: np.asarray(v, dtype=np.float32)
              for k, v in inputs.items() if k not in ("x", "kmeans_label")}

    if mode == "bf16":
        strat = _route_structured(labels)
        if strat is not None:
            return _run_structured(x, params, strat, trace)
    return _run_generic(x, params, labels, mode, trace)


def kernel(**inputs):
    out, _ = kernel_traced(inputs, trace=False)
    return out
